# revision 11
# baseline (speedup 1.0000x reference)
"""Trainium2 Bass kernel for nn_MemoryEfficientAttention (full MHA).

Reference computation (fp32):
    q = split_heads(x @ Wq.T + bq); k, v likewise
    attn = softmax(q @ k.T / sqrt(64))
    out = merge_heads(attn @ v) @ Wo.T + bo

Shapes: B=2, S=4096, D=1024, H=16, head_dim=64.

Sharding across 8 NeuronCores (Megatron attention sharding):
  - 2 heads per core (= 128 of the 1024 projection dims, contiguous slice).
  - Q/K/V projections column-parallel, output projection row-parallel;
    the 8 per-core partial outputs are summed on the host (+ bo).
  - bv never enters the device: softmax rows sum to 1, so its entire effect
    on the output is the constant vector Wo @ bv, added on the host.

Per-core kernel (flash-attention style, nothing S^2-sized touches HBM):
  The kernel is ScalarE-bound: the exp of the S^2 score matrix costs
  ~1.11us per (q-chunk, key-tile) step and 512 such steps = 569us.  All
  emission is therefore woven so ScalarE never starves: projection
  matmul chains are split in half and spread between attention j-steps
  (one chain-half per few steps), never as multi-us blocks.  xt is
  4-deep buffered so input DMA is never gated by tile release.
  Per j-step: scoresT for both heads run concurrently on row-split PE
  tiles, exp on ScalarE (no max subtraction -- scores are bounded), fp16
  PV matmul with an interleaved ones-column accumulating the softmax
  denominator.  Raw output + reciprocal of the denominator leave PSUM
  immediately; normalization resolves during the next chunk.  Output
  projection for chunk qc is woven into chunk qc+1's steps.
"""

import sys

if "/opt/trn_rl_repo" not in sys.path:
    sys.path.insert(0, "/opt/trn_rl_repo")

import numpy as np

B = 2
S_FULL = 4096
D = 1024
H = 16
HD = 64
NCORES = 8
DC = 128          # head dims per core (2 heads x 64)
SCALE = 1.0 / 8.0  # 1/sqrt(64)


def build_kernel(S=S_FULL):
    """Build the per-core Bass program. Returns the compiled Bacc object."""
    import concourse.bacc as bacc
    import concourse.tile as tile
    from concourse import mybir

    f32 = mybir.dt.float32
    f16 = mybir.dt.float16
    AF = mybir.ActivationFunctionType

    KT = D // 128       # k-tiles over the projection contraction dim
    SQ = 512            # q-chunk size
    NQC = S // SQ       # q chunks per batch
    NKT = S // 128      # key tiles per batch
    NM = S // 512       # x chunks for projections

    nc = bacc.Bacc("TRN2", target_bir_lowering=False, debug=False,
                   num_devices=NCORES)

    xT = nc.dram_tensor("xT", [B, D, S], f16, kind="ExternalInput").ap()
    wqT = nc.dram_tensor("wqT", [D, DC], f16, kind="ExternalInput").ap()
    wkT = nc.dram_tensor("wkT", [D, DC], f16, kind="ExternalInput").ap()
    wvT = nc.dram_tensor("wvT", [D, DC], f16, kind="ExternalInput").ap()
    woT = nc.dram_tensor("woT", [DC, D], f16, kind="ExternalInput").ap()
    bq = nc.dram_tensor("bq", [DC], f32, kind="ExternalInput").ap()
    part = nc.dram_tensor("part", [B, S, D], f32, kind="ExternalOutput").ap()

    with tile.TileContext(nc) as tc:
        with (
            tc.tile_pool(name="consts", bufs=1) as consts,
            tc.tile_pool(name="xt", bufs=4) as xt_pool,
            tc.tile_pool(name="qkv", bufs=2) as qkv_pool,
            tc.tile_pool(name="exp", bufs=3) as exp_pool,
            tc.tile_pool(name="att", bufs=2) as att_pool,
            tc.tile_pool(name="small", bufs=4) as small_pool,
            tc.tile_pool(name="bounce", bufs=2, space="DRAM") as dram_pool,
            tc.tile_pool(name="ps_scores", bufs=2, space="PSUM") as ps_scores,
            tc.tile_pool(name="ps_acc", bufs=2, space="PSUM") as ps_acc,
            tc.tile_pool(name="ps_shared", bufs=2, space="PSUM") as ps_shared,
        ):
            # ---- constants ----
            wq_sb = consts.tile([128, KT, DC], f16)
            wk_sb = consts.tile([128, KT, DC], f16)
            wv_sb = consts.tile([128, KT, DC], f16)
            wo_sb = consts.tile([128, D], f16)
            bq_sb = consts.tile([128, 1], f32)

            # tiny dummy exp so the ACT table set loads during startup DMAs
            warm = consts.tile([128, 1], f32)
            nc.vector.memset(warm[:], 0.0)
            nc.scalar.activation(warm[:], warm[:], AF.Exp, scale=1.0)

            # k/q weights + bias first (first proj chains need them);
            # wv next, wo last (only needed ~30us in).  bk is dropped
            # entirely: it shifts every score by (q.bk)/8, constant along
            # the key axis, which cancels in the softmax.
            nc.sync.dma_start(
                out=wk_sb[:], in_=wkT.rearrange("(kt p) m -> p kt m", p=128))
            nc.sync.dma_start(
                out=wq_sb[:], in_=wqT.rearrange("(kt p) m -> p kt m", p=128))
            nc.sync.dma_start(
                out=bq_sb[:], in_=bq.rearrange("(p o) -> p o", o=1))
            nc.gpsimd.dma_start(
                out=wv_sb[:], in_=wvT.rearrange("(kt p) m -> p kt m", p=128))
            nc.gpsimd.dma_start(out=wo_sb[:], in_=woT)

            state = [None, None]  # per-batch dict of tiles

            def alloc_batch(b):
                qT_sb = qkv_pool.tile([128, S], f16, tag="qT", name=f"qT_{b}")
                kT_sb = qkv_pool.tile([128, S], f16, tag="kT", name=f"kT_{b}")
                # v natural layout, per key-tile: [vA(64) | 1 | vB(64) | 1]
                v_sb = qkv_pool.tile([128, NKT, 130], f16, tag="v",
                                     name=f"v_{b}")
                nc.vector.memset(v_sb[:, :, 64:65], 1.0)
                nc.vector.memset(v_sb[:, :, 129:130], 1.0)
                attT_sb = att_pool.tile([128, S], f16, tag="attT",
                                        name=f"attT_{b}")
                den_dram = dram_pool.tile([2, S], f16, tag="den",
                                          name=f"den_{b}")
                state[b] = dict(qT=qT_sb, kT=kT_sb, v=v_sb, attT=attT_sb,
                                den=den_dram)

            def proj_dma(b, m):
                """Allocate the x tile for chunk m and return its DMA item.
                Call order must match usage order (per-tag ring)."""
                xt = xt_pool.tile([128, KT, 512], f16, tag="xt",
                                  name=f"xt_{b}_{m}")

                def run():
                    xsrc = xT[b][:, m * 512:(m + 1) * 512].rearrange(
                        "(kt p) s -> p kt s", p=128)
                    half = KT // 2
                    nc.sync.dma_start(out=xt[:, 0:half, :],
                                      in_=xsrc[:, 0:half, :])
                    nc.gpsimd.dma_start(out=xt[:, half:, :],
                                        in_=xsrc[:, half:, :])
                return xt, run

            def proj_chains(b, m, xt):
                """Projection matmul chains for chunk m as 8 small items
                (each <=1us of PE). PSUM tiles alloc at emission time so
                the shared-ring order matches usage order."""
                st = state[b]
                items = []

                def qk_chain(w_sb, b_sb, dst):
                    cell = {}

                    def half1():
                        cell["ps"] = ps_shared.tile(
                            [128, 512], f32, tag="shared",
                            name=f"ps_{b}_{m}_{id(w_sb)}")
                        for j in range(KT // 2):
                            nc.tensor.matmul(
                                cell["ps"][:], lhsT=w_sb[:, j, :],
                                rhs=xt[:, j, :],
                                start=(j == 0), stop=False)

                    def half2():
                        ps = cell["ps"]
                        for j in range(KT // 2, KT):
                            nc.tensor.matmul(
                                ps[:], lhsT=w_sb[:, j, :], rhs=xt[:, j, :],
                                start=False, stop=(j == KT - 1))
                        dslice = dst[:, m * 512:(m + 1) * 512]
                        if b_sb is None:
                            nc.vector.tensor_copy(dslice, ps[:])
                        else:
                            nc.vector.tensor_scalar_add(dslice, ps[:], b_sb[:])
                    return [half1, half2]

                items += qk_chain(wk_sb, None, st["kT"])
                items += qk_chain(wq_sb, bq_sb, st["qT"])

                # V in natural layout: x-tile stationary, Wv moving.
                def v_item(t):
                    def run():
                        psv = ps_shared.tile([128, 512], f32, tag="shared",
                                             name=f"psv_{b}_{m}_{t}")
                        for j in range(KT):
                            nc.tensor.matmul(
                                psv[:, 0:DC],
                                lhsT=xt[:, j, t * 128:(t + 1) * 128],
                                rhs=wv_sb[:, j, :],
                                start=(j == 0), stop=(j == KT - 1))
                        kt_idx = m * 4 + t
                        nc.vector.tensor_copy(
                            st["v"][:, kt_idx, 0:64], psv[:, 0:64])
                        nc.vector.tensor_copy(
                            st["v"][:, kt_idx, 65:129], psv[:, 64:128])
                    return run

                items += [v_item(t) for t in range(4)]
                return items

            def attn_steps(b, qc):
                """One q-chunk of 512 rows: 32 j-step closures."""
                st = state[b]
                qT_sb, kT_sb, v_sb = st["qT"], st["kT"], st["v"]
                q0, q1 = qc * SQ, (qc + 1) * SQ
                ctx = {}

                def step(j):
                    def run():
                        if j == 0:
                            ctx["acc"] = [
                                ps_acc.tile([128, SQ], f32, tag="acc",
                                            name=f"acc{hh}_{b}_{qc}")
                                for hh in range(2)]
                        accs = ctx["acc"]
                        k0, k1 = j * 128, (j + 1) * 128
                        pss = ps_scores.tile([128, 2 * SQ], f32, tag="scores",
                                             name=f"pss_{b}_{qc}_{j}")
                        for hh in range(2):
                            nc.tensor.matmul(
                                pss[:, hh * SQ:(hh + 1) * SQ],
                                lhsT=kT_sb[hh * 64:(hh + 1) * 64, k0:k1],
                                rhs=qT_sb[hh * 64:(hh + 1) * 64, q0:q1],
                                start=True, stop=True)
                        ex = exp_pool.tile([128, 2 * SQ], f16, tag="exp",
                                           name=f"ex_{b}_{qc}_{j}")
                        nc.scalar.activation(ex[:], pss[:], AF.Exp,
                                             scale=SCALE)
                        for hh in range(2):
                            nc.tensor.matmul(
                                accs[hh][0:65, :],
                                lhsT=v_sb[:, j, hh * 65:(hh + 1) * 65],
                                rhs=ex[:, hh * SQ:(hh + 1) * SQ],
                                start=(j == 0), stop=(j == NKT - 1))
                    return run

                return [step(j) for j in range(NKT)], ctx

            def attn_fin(b, qc, ctx):
                """PSUM release + softmax normalization for a q-chunk."""
                st = state[b]
                attT_sb, den_dram = st["attT"], st["den"]
                q0, q1 = qc * SQ, (qc + 1) * SQ
                accs = ctx["acc"]
                for hh in range(2):
                    nc.vector.tensor_copy(
                        attT_sb[hh * 64:(hh + 1) * 64, q0:q1],
                        accs[hh][0:64, :])
                    dn = small_pool.tile([1, SQ], f32, tag="dn",
                                         name=f"dn_{b}_{qc}_{hh}")
                    nc.vector.tensor_copy(dn[:], accs[hh][64:65, :])
                    rcp = small_pool.tile([1, SQ], f32, tag="rcp",
                                          name=f"rcp_{b}_{qc}_{hh}")
                    nc.vector.reciprocal_approx_fast(rcp[:], dn[:])
                    rcp16 = small_pool.tile([1, SQ], f16, tag="rcp16",
                                            name=f"rcp16_{b}_{qc}_{hh}")
                    with nc.allow_low_precision(reason="fp16 softmax recip"):
                        nc.vector.tensor_copy(rcp16[:], rcp[:])
                    nc.gpsimd.dma_start(out=den_dram[hh, q0:q1], in_=rcp16[:])

                # broadcast the reciprocal + normalize; resolves during the
                # next chunk's attention
                for hh in range(2):
                    bc = small_pool.tile([128, SQ], f16, tag="bcast",
                                         name=f"bc_{b}_{qc}_{hh}")
                    bch = bc[hh * 64:(hh + 1) * 64, :]
                    rd = den_dram[hh, q0:q1]
                    bcast_src = rd.__class__(
                        tensor=rd.tensor, offset=rd.offset,
                        ap=[[0, 64]] + list(rd.ap),
                    )
                    nc.gpsimd.dma_start(out=bch, in_=bcast_src)
                    nc.vector.tensor_mul(
                        attT_sb[hh * 64:(hh + 1) * 64, q0:q1],
                        attT_sb[hh * 64:(hh + 1) * 64, q0:q1],
                        bch,
                    )

            def tail_items(b, qc, engines=("vector",)):
                """Output projection for a q-chunk as 8 single-tile items,
                emitted one chunk late so the normalization chain has
                resolved and the in-order PE never stalls on it."""
                st = state[b]
                attT_sb = st["attT"]
                q0 = qc * SQ
                items = []
                for sti in range(SQ // 128):
                    for oc in range(D // 512):
                        def run(sti=sti, oc=oc, n=sti * 2 + oc):
                            s0 = q0 + sti * 128
                            s1 = s0 + 128
                            pso = ps_shared.tile(
                                [128, 512], f32, tag="shared",
                                name=f"pso_{b}_{qc}_{sti}_{oc}")
                            nc.tensor.matmul(
                                pso[:], lhsT=attT_sb[:, s0:s1],
                                rhs=wo_sb[:, oc * 512:(oc + 1) * 512],
                                start=True, stop=True)
                            ob = small_pool.tile([128, 512], f32, tag="ob",
                                                 name=f"ob_{b}_{qc}_{sti}_{oc}")
                            eng = engines[n % len(engines)]
                            if eng == "scalar":
                                nc.scalar.copy(ob[:], pso[:])
                            else:
                                nc.vector.tensor_copy(ob[:], pso[:])
                            nc.sync.dma_start(
                                out=part[b, s0:s1, oc * 512:(oc + 1) * 512],
                                in_=ob[:])
                        items.append(run)
                return items

            def weave(steps, extras):
                """Run all steps in order, spreading extras evenly between
                them (extras trail: first extra after the first step)."""
                n, e = len(steps), len(extras)
                ei = 0
                for i, s in enumerate(steps):
                    s()
                    while ei < e and (ei + 1) * n <= (i + 1) * e:
                        extras[ei]()
                        ei += 1
                while ei < e:
                    extras[ei]()
                    ei += 1

            # ---- emission schedule ----
            # Phase A: batch-0 chunk 0 (DMA + chains up front, chunk 1 DMA
            # prefetched), then chunks 1..7 woven with the first q-chunk's
            # attention steps (4 steps per chunk), always one DMA ahead.
            alloc_batch(0)
            xts = {}
            for m in range(2):
                xts[m], dma = proj_dma(0, m)
                dma()
            for it in proj_chains(0, 0, xts[0]):
                it()
            qc0_steps, qc0_ctx = attn_steps(0, 0)
            idx = 0
            for m in range(1, NM):
                extras = []
                if m + 1 < NM:
                    xts[m + 1], dma = proj_dma(0, m + 1)
                    extras.append(dma)
                extras += proj_chains(0, m, xts[m])
                weave(qc0_steps[idx:idx + 4], extras)
                idx += 4
            for s in qc0_steps[idx:]:
                s()
            attn_fin(0, 0, qc0_ctx)
            pending = [(0, 0)]

            # Phase B: batch-0 q-chunks 1..7; batch-1 projections woven in
            # (spread over qc 1..6, chunk DMAs one chunk ahead), tails of
            # qc-1 woven too.
            b1_items = []
            for qc in range(1, NQC):
                steps, ctx = attn_steps(0, qc)
                if qc == 1:
                    alloc_batch(1)
                    b1_xts = {}
                    b1_xts[0], dma0 = proj_dma(1, 0)
                    b1_items.append(dma0)
                    for m in range(NM):
                        if m + 1 < NM:
                            b1_xts[m + 1], dma = proj_dma(1, m + 1)
                            b1_items.append(dma)
                        b1_items += proj_chains(1, m, b1_xts[m])
                    per_qc = (len(b1_items) + NQC - 3) // (NQC - 2)
                extras = []
                if pending:
                    extras += tail_items(*pending.pop(0))
                if b1_items:
                    take, b1_items = b1_items[:per_qc], b1_items[per_qc:]
                    extras += take
                weave(steps, extras)
                attn_fin(0, qc, ctx)
                pending.append((0, qc))

            # Phase C: batch-1 attention.
            for qc in range(NQC):
                steps, ctx = attn_steps(1, qc)
                extras = []
                if pending:
                    extras += tail_items(*pending.pop(0))
                weave(steps, extras)
                attn_fin(1, qc, ctx)
                pending.append((1, qc))

            # Final drain: ScalarE is idle now; alternate copy engines and
            # let ps_shared double-buffer the pso tiles.
            while pending:
                for it in tail_items(*pending.pop(0),
                                     engines=("scalar", "vector")):
                    it()

    nc.compile()
    return nc


def shard_inputs(x, Wq, bq, Wk, bk, Wv, bv, Wo, bo, S=S_FULL):
    """Host-side sharding: returns list of 8 per-core input dicts."""
    x = np.asarray(x, dtype=np.float32)
    xT = np.ascontiguousarray(x.transpose(0, 2, 1)).astype(np.float16)  # [B, D, S]
    in_maps = []
    for c in range(NCORES):
        sl = slice(c * DC, (c + 1) * DC)
        in_maps.append({
            "xT": xT,
            "wqT": np.ascontiguousarray(np.asarray(Wq)[sl, :].T).astype(np.float16),
            "wkT": np.ascontiguousarray(np.asarray(Wk)[sl, :].T).astype(np.float16),
            "wvT": np.ascontiguousarray(np.asarray(Wv)[sl, :].T).astype(np.float16),
            "woT": np.ascontiguousarray(np.asarray(Wo)[:, sl].T).astype(np.float16),
            "bq": np.ascontiguousarray(np.asarray(bq)[sl], dtype=np.float32),
        })
    return in_maps


_NC_CACHE = {}


def _get_nc(S=S_FULL):
    if S not in _NC_CACHE:
        _NC_CACHE[S] = build_kernel(S)
    return _NC_CACHE[S]


def kernel(x, Wq, bq, Wk, bk, Wv, bv, Wo, bo, _trace=False, _trace_cores=None):
    from concourse import bass_utils

    nc = _get_nc(S_FULL)
    in_maps = shard_inputs(x, Wq, bq, Wk, bk, Wv, bv, Wo, bo)
    kwargs = {}
    if _trace:
        kwargs = dict(trace=True, trace_cores=_trace_cores or [0])
    res = bass_utils.run_bass_kernel_spmd(
        nc, in_maps, core_ids=list(range(NCORES)), **kwargs)
    out = np.zeros((B, S_FULL, D), dtype=np.float32)
    for c in range(NCORES):
        out += res.results[c]["part"]
    # bv is folded out of the device kernel: softmax rows sum to one, so its
    # contribution to the output is the constant Wo @ bv. Add it with bo here.
    bias = (np.asarray(Wo, dtype=np.float64) @ np.asarray(bv, dtype=np.float64)
            + np.asarray(bo, dtype=np.float64))
    out += bias.astype(np.float32)[None, None, :]
    if _trace:
        kernel._last_results = res
    return out


# revision 16
# speedup vs baseline: 1.0705x; 1.0705x over previous
"""Trainium2 Bass kernel for nn_MemoryEfficientAttention (full MHA).

Reference computation (fp32):
    q = split_heads(x @ Wq.T + bq); k, v likewise
    attn = softmax(q @ k.T / sqrt(64))
    out = merge_heads(attn @ v) @ Wo.T + bo

Shapes: B=2, S=4096, D=1024, H=16, head_dim=64.

Sharding across 8 NeuronCores (Megatron attention sharding):
  - 2 heads per core (= 128 of the 1024 projection dims, contiguous slice).
  - Q/K/V projections column-parallel, output projection row-parallel;
    the 8 per-core partial outputs are summed on the host (+ bo).
  - bv never enters the device: softmax rows sum to 1, so its entire effect
    on the output is the constant vector Wo @ bv, added on the host.

Per-core kernel (flash-attention style, nothing S^2-sized touches HBM):
  The kernel is ScalarE-bound: the exp of the S^2 score matrix costs
  ~1.11us per (q-chunk, key-tile) step and 512 such steps = 569us.  All
  emission is therefore woven so ScalarE never starves: projection
  matmul chains are split in half and spread between attention j-steps
  (one chain-half per few steps), never as multi-us blocks.  xt is
  4-deep buffered so input DMA is never gated by tile release.
  Per j-step: scoresT for both heads run concurrently on row-split PE
  tiles, exp on ScalarE (no max subtraction -- scores are bounded), fp16
  PV matmul with an interleaved ones-column accumulating the softmax
  denominator.  Raw output + reciprocal of the denominator leave PSUM
  immediately; normalization resolves during the next chunk.  Output
  projection for chunk qc is woven into chunk qc+1's steps.
"""

import sys

if "/opt/trn_rl_repo" not in sys.path:
    sys.path.insert(0, "/opt/trn_rl_repo")

import numpy as np

B = 2
S_FULL = 4096
D = 1024
H = 16
HD = 64
NCORES = 8
DC = 128          # head dims per core (2 heads x 64)
SCALE = 1.0 / 8.0  # 1/sqrt(64)


def build_kernel(S=S_FULL):
    """Build the per-core Bass program. Returns the compiled Bacc object."""
    import concourse.bacc as bacc
    import concourse.tile as tile
    from concourse import mybir

    f32 = mybir.dt.float32
    f16 = mybir.dt.float16
    AF = mybir.ActivationFunctionType

    KT = D // 128       # k-tiles over the projection contraction dim
    SQ = 512            # q-chunk size
    NQC = S // SQ       # q chunks per batch
    NKT = S // 128      # key tiles per batch
    NM = S // 512       # x chunks for projections

    nc = bacc.Bacc("TRN2", target_bir_lowering=False, debug=False,
                   num_devices=NCORES)

    xT = nc.dram_tensor("xT", [B, D, S], f16, kind="ExternalInput").ap()
    wqT = nc.dram_tensor("wqT", [D, DC], f16, kind="ExternalInput").ap()
    wkT = nc.dram_tensor("wkT", [D, DC], f16, kind="ExternalInput").ap()
    wvT = nc.dram_tensor("wvT", [D, DC], f16, kind="ExternalInput").ap()
    woT = nc.dram_tensor("woT", [DC, D], f16, kind="ExternalInput").ap()
    bq = nc.dram_tensor("bq", [DC], f32, kind="ExternalInput").ap()
    part = nc.dram_tensor("part", [B, S, D], f32, kind="ExternalOutput").ap()

    with tile.TileContext(nc) as tc:
        with (
            tc.tile_pool(name="consts", bufs=1) as consts,
            tc.tile_pool(name="xt", bufs=4) as xt_pool,
            tc.tile_pool(name="qkv", bufs=2) as qkv_pool,
            tc.tile_pool(name="exp", bufs=4) as exp_pool,
            tc.tile_pool(name="exp_pf", bufs=8) as exp_pf_pool,
            tc.tile_pool(name="att", bufs=2) as att_pool,
            tc.tile_pool(name="small", bufs=4) as small_pool,
            tc.tile_pool(name="bounce", bufs=2, space="DRAM") as dram_pool,
            tc.tile_pool(name="ps_scores", bufs=2, space="PSUM") as ps_scores,
            tc.tile_pool(name="ps_acc", bufs=2, space="PSUM") as ps_acc,
            tc.tile_pool(name="ps_shared", bufs=2, space="PSUM") as ps_shared,
        ):
            # ---- constants ----
            wq_sb = consts.tile([128, KT, DC], f16)
            wk_sb = consts.tile([128, KT, DC], f16)
            wv_sb = consts.tile([128, KT, DC], f16)
            wo_sb = consts.tile([128, D], f16)
            bq_sb = consts.tile([128, 1], f32)

            # tiny dummy exp so the ACT table set loads during startup DMAs
            warm = consts.tile([128, 1], f32)
            nc.vector.memset(warm[:], 0.0)
            nc.scalar.activation(warm[:], warm[:], AF.Exp, scale=1.0)

            # k/q weights + bias first (first proj chains need them);
            # wv next, wo last (only needed ~30us in).  bk is dropped
            # entirely: it shifts every score by (q.bk)/8, constant along
            # the key axis, which cancels in the softmax.
            nc.sync.dma_start(
                out=wk_sb[:], in_=wkT.rearrange("(kt p) m -> p kt m", p=128))
            nc.sync.dma_start(
                out=wq_sb[:], in_=wqT.rearrange("(kt p) m -> p kt m", p=128))
            nc.sync.dma_start(
                out=bq_sb[:], in_=bq.rearrange("(p o) -> p o", o=1))
            nc.gpsimd.dma_start(
                out=wv_sb[:], in_=wvT.rearrange("(kt p) m -> p kt m", p=128))
            nc.gpsimd.dma_start(out=wo_sb[:], in_=woT)

            state = [None, None]  # per-batch dict of tiles

            def alloc_batch(b):
                qT_sb = qkv_pool.tile([128, S], f16, tag="qT", name=f"qT_{b}")
                kT_sb = qkv_pool.tile([128, S], f16, tag="kT", name=f"kT_{b}")
                # v natural layout, per key-tile: [vA(64) | 1 | vB(64) | 1]
                v_sb = qkv_pool.tile([128, NKT, 130], f16, tag="v",
                                     name=f"v_{b}")
                nc.vector.memset(v_sb[:, :, 64:65], 1.0)
                nc.vector.memset(v_sb[:, :, 129:130], 1.0)
                attT_sb = att_pool.tile([128, S], f16, tag="attT",
                                        name=f"attT_{b}")
                den_dram = dram_pool.tile([2, S], f16, tag="den",
                                          name=f"den_{b}")
                state[b] = dict(qT=qT_sb, kT=kT_sb, v=v_sb, attT=attT_sb,
                                den=den_dram)

            def proj_dma(b, m):
                """Allocate the x tile for chunk m and return its DMA item.
                Call order must match usage order (per-tag ring)."""
                xt = xt_pool.tile([128, KT, 512], f16, tag="xt",
                                  name=f"xt_{b}_{m}")

                def run():
                    xsrc = xT[b][:, m * 512:(m + 1) * 512].rearrange(
                        "(kt p) s -> p kt s", p=128)
                    half = KT // 2
                    nc.sync.dma_start(out=xt[:, 0:half, :],
                                      in_=xsrc[:, 0:half, :])
                    nc.gpsimd.dma_start(out=xt[:, half:, :],
                                        in_=xsrc[:, half:, :])
                return xt, run

            def proj_chains(b, m, xt):
                """Projection matmul chains for chunk m as 8 small items
                (each <=1us of PE). PSUM tiles alloc at emission time so
                the shared-ring order matches usage order."""
                st = state[b]
                items = []

                def qk_chain(w_sb, b_sb, dst):
                    cell = {}

                    def half1():
                        cell["ps"] = ps_shared.tile(
                            [128, 512], f32, tag="shared",
                            name=f"ps_{b}_{m}_{id(w_sb)}")
                        for j in range(KT // 2):
                            nc.tensor.matmul(
                                cell["ps"][:], lhsT=w_sb[:, j, :],
                                rhs=xt[:, j, :],
                                start=(j == 0), stop=False)

                    def half2():
                        ps = cell["ps"]
                        for j in range(KT // 2, KT):
                            nc.tensor.matmul(
                                ps[:], lhsT=w_sb[:, j, :], rhs=xt[:, j, :],
                                start=False, stop=(j == KT - 1))
                        dslice = dst[:, m * 512:(m + 1) * 512]
                        if b_sb is None:
                            nc.vector.tensor_copy(dslice, ps[:])
                        else:
                            nc.vector.tensor_scalar_add(dslice, ps[:], b_sb[:])
                    return [half1, half2]

                items += qk_chain(wk_sb, None, st["kT"])
                items += qk_chain(wq_sb, bq_sb, st["qT"])

                # V in natural layout: x-tile stationary, Wv moving.
                def v_item(t):
                    def run():
                        psv = ps_shared.tile([128, 512], f32, tag="shared",
                                             name=f"psv_{b}_{m}_{t}")
                        for j in range(KT):
                            nc.tensor.matmul(
                                psv[:, 0:DC],
                                lhsT=xt[:, j, t * 128:(t + 1) * 128],
                                rhs=wv_sb[:, j, :],
                                start=(j == 0), stop=(j == KT - 1))
                        kt_idx = m * 4 + t
                        nc.vector.tensor_copy(
                            st["v"][:, kt_idx, 0:64], psv[:, 0:64])
                        nc.vector.tensor_copy(
                            st["v"][:, kt_idx, 65:129], psv[:, 64:128])
                    return run

                items += [v_item(t) for t in range(4)]
                return items

            def attn_parts(b, qc, pf_tag=None):
                """One q-chunk of 512 rows, split into scores+exp parts and
                AV parts so emission can skew AV behind scores (the first
                AV of a chunk waits on the acc-bank release; skewing keeps
                it from blocking ready scores in the in-order PE queue).
                pf_tag: use a dedicated deep ex ring (phase-A prefetch)."""
                st = state[b]
                qT_sb, kT_sb, v_sb = st["qT"], st["kT"], st["v"]
                q0, q1 = qc * SQ, (qc + 1) * SQ
                ctx = {"ex": {}}

                def sc_part(j):
                    def run():
                        k0, k1 = j * 128, (j + 1) * 128
                        pss = ps_scores.tile([128, 2 * SQ], f32, tag="scores",
                                             name=f"pss_{b}_{qc}_{j}")
                        for hh in range(2):
                            nc.tensor.matmul(
                                pss[:, hh * SQ:(hh + 1) * SQ],
                                lhsT=kT_sb[hh * 64:(hh + 1) * 64, k0:k1],
                                rhs=qT_sb[hh * 64:(hh + 1) * 64, q0:q1],
                                start=True, stop=True)
                        pool = exp_pf_pool if (pf_tag and j < 8) else exp_pool
                        ex = pool.tile([128, 2 * SQ], f16,
                                       tag="exp_pf" if (pf_tag and j < 8)
                                       else "exp",
                                       name=f"ex_{b}_{qc}_{j}")
                        nc.scalar.activation(ex[:], pss[:], AF.Exp,
                                             scale=SCALE)
                        ctx["ex"][j] = ex
                    return run

                def av_part(j):
                    def run():
                        if j == 0:
                            ctx["acc"] = [
                                ps_acc.tile([128, SQ], f32, tag="acc",
                                            name=f"acc{hh}_{b}_{qc}")
                                for hh in range(2)]
                        accs = ctx["acc"]
                        ex = ctx["ex"].pop(j)
                        for hh in range(2):
                            nc.tensor.matmul(
                                accs[hh][0:65, :],
                                lhsT=v_sb[:, j, hh * 65:(hh + 1) * 65],
                                rhs=ex[:, hh * SQ:(hh + 1) * SQ],
                                start=(j == 0), stop=(j == NKT - 1))
                    return run

                return ([sc_part(j) for j in range(NKT)],
                        [av_part(j) for j in range(NKT)], ctx)

            def attn_fin(b, qc, ctx):
                """PSUM release + softmax normalization for a q-chunk."""
                st = state[b]
                attT_sb, den_dram = st["attT"], st["den"]
                q0, q1 = qc * SQ, (qc + 1) * SQ
                accs = ctx["acc"]
                for hh in range(2):
                    nc.vector.tensor_copy(
                        attT_sb[hh * 64:(hh + 1) * 64, q0:q1],
                        accs[hh][0:64, :])
                    dn = small_pool.tile([1, SQ], f32, tag="dn",
                                         name=f"dn_{b}_{qc}_{hh}")
                    nc.vector.tensor_copy(dn[:], accs[hh][64:65, :])
                    rcp = small_pool.tile([1, SQ], f32, tag="rcp",
                                          name=f"rcp_{b}_{qc}_{hh}")
                    nc.vector.reciprocal_approx_fast(rcp[:], dn[:])
                    rcp16 = small_pool.tile([1, SQ], f16, tag="rcp16",
                                            name=f"rcp16_{b}_{qc}_{hh}")
                    with nc.allow_low_precision(reason="fp16 softmax recip"):
                        nc.vector.tensor_copy(rcp16[:], rcp[:])
                    nc.gpsimd.dma_start(out=den_dram[hh, q0:q1], in_=rcp16[:])

                # broadcast the reciprocal + normalize; resolves during the
                # next chunk's attention
                for hh in range(2):
                    bc = small_pool.tile([128, SQ], f16, tag="bcast",
                                         name=f"bc_{b}_{qc}_{hh}")
                    bch = bc[hh * 64:(hh + 1) * 64, :]
                    rd = den_dram[hh, q0:q1]
                    bcast_src = rd.__class__(
                        tensor=rd.tensor, offset=rd.offset,
                        ap=[[0, 64]] + list(rd.ap),
                    )
                    nc.gpsimd.dma_start(out=bch, in_=bcast_src)
                    nc.vector.tensor_mul(
                        attT_sb[hh * 64:(hh + 1) * 64, q0:q1],
                        attT_sb[hh * 64:(hh + 1) * 64, q0:q1],
                        bch,
                    )

            def tail_items(b, qc, engines=("vector",)):
                """Output projection for a q-chunk as 8 single-tile items,
                emitted one chunk late so the normalization chain has
                resolved and the in-order PE never stalls on it."""
                st = state[b]
                attT_sb = st["attT"]
                q0 = qc * SQ
                items = []
                for sti in range(SQ // 128):
                    for oc in range(D // 512):
                        def run(sti=sti, oc=oc, n=sti * 2 + oc):
                            s0 = q0 + sti * 128
                            s1 = s0 + 128
                            pso = ps_shared.tile(
                                [128, 512], f32, tag="shared",
                                name=f"pso_{b}_{qc}_{sti}_{oc}")
                            nc.tensor.matmul(
                                pso[:], lhsT=attT_sb[:, s0:s1],
                                rhs=wo_sb[:, oc * 512:(oc + 1) * 512],
                                start=True, stop=True)
                            ob = small_pool.tile([128, 512], f32, tag="ob",
                                                 name=f"ob_{b}_{qc}_{sti}_{oc}")
                            eng = engines[n % len(engines)]
                            if eng == "scalar":
                                nc.scalar.copy(ob[:], pso[:])
                            else:
                                nc.vector.tensor_copy(ob[:], pso[:])
                            nc.sync.dma_start(
                                out=part[b, s0:s1, oc * 512:(oc + 1) * 512],
                                in_=ob[:])
                        items.append(run)
                return items

            def weave(steps, extras):
                """Run all steps in order, spreading extras evenly between
                them (extras trail: first extra after the first step)."""
                n, e = len(steps), len(extras)
                ei = 0
                for i, s in enumerate(steps):
                    s()
                    while ei < e and (ei + 1) * n <= (i + 1) * e:
                        extras[ei]()
                        ei += 1
                while ei < e:
                    extras[ei]()
                    ei += 1

            SKEW = 2  # slots AV lags scores within a chunk

            def qc_slots(sc, av, start=0):
                """Slot closures for one chunk: sc[j] at slot j, av[j]
                lagging SKEW slots, trailing AVs at the end."""
                slots = []
                for j in range(start, NKT):
                    def slot(j=j):
                        sc[j]()
                        if j - SKEW >= start:
                            av[j - SKEW]()
                    slots.append(slot)
                for j in range(NKT - SKEW, NKT):
                    slots.append(av[j])
                return slots

            # ---- emission schedule ----
            # Phase A: batch-0 chunk-0 k/q chains up front, then chunks
            # 1..7 woven with the first q-chunk's slots (4 per chunk, one
            # chunk-DMA ahead).  qc1's first PF score+exp parts are
            # prefetched into phase A (deep ex ring) so ScalarE has more
            # than one chunk's exp work while PE grinds projections.
            PF = 8
            alloc_batch(0)
            xts = {}
            for m in range(2):
                xts[m], dma = proj_dma(0, m)
                dma()
            ch0 = proj_chains(0, 0, xts[0])
            for it in ch0[:4]:
                it()
            sc0, av0, ctx0 = attn_parts(0, 0)
            sc1, av1, ctx1 = attn_parts(0, 1, pf_tag="exp_pf")
            pf_sched = {2: [0, 1], 3: [2, 3], 4: [4], 5: [5], 6: [6], 7: [7]}

            def slot_a(j):
                def run():
                    sc0[j]()
                    if j >= SKEW:
                        av0[j - SKEW]()
                return run

            for m in range(1, NM):
                extras = []
                if m == 1:
                    extras += ch0[4:]
                if m + 1 < NM:
                    xts[m + 1], dma = proj_dma(0, m + 1)
                    extras.append(dma)
                extras += proj_chains(0, m, xts[m])
                extras += [sc1[p] for p in pf_sched.get(m, [])]
                weave([slot_a(j) for j in range(4 * (m - 1), 4 * m)], extras)
            for j in range(4 * (NM - 1), NKT):
                slot_a(j)()
            for j in range(NKT - SKEW, NKT):
                av0[j]()
            attn_fin(0, 0, ctx0)
            pending = [(0, 0)]

            # Phase B: batch-0 qc1 (AV catch-up for the prefetched parts),
            # then qc 2..7 with batch-1 projections woven over qc 2..6 and
            # tails of qc-1 woven throughout.
            def qc1_slots():
                # sc j>=8 uses the normal ex ring (4): sc[j] allocation
                # waits on av[j-4], so every av[j-4] must be emitted before
                # sc[j].  av[0..7] read the deep prefetch ring.
                slots = [sc1[8], sc1[9], av1[0], sc1[10], av1[1],
                         sc1[11], av1[2]]
                slots += [av1[j] for j in range(3, 9)]   # catch up to av8
                for j in range(12, NKT):
                    slots.append(sc1[j])
                    slots.append(av1[j - 3])
                slots += [av1[j] for j in range(NKT - 3, NKT)]
                return slots

            b1_items = []
            for qc in range(1, NQC):
                if qc == 1:
                    slots = qc1_slots()
                    ctx = ctx1
                else:
                    sc, av, ctx = attn_parts(0, qc)
                    slots = qc_slots(sc, av)
                if qc == 2:
                    alloc_batch(1)
                    b1_xts = {}
                    b1_xts[0], dma0 = proj_dma(1, 0)
                    b1_items.append(dma0)
                    for m in range(NM):
                        if m + 1 < NM:
                            b1_xts[m + 1], dma = proj_dma(1, m + 1)
                            b1_items.append(dma)
                        b1_items += proj_chains(1, m, b1_xts[m])
                    per_qc = (len(b1_items) + NQC - 4) // (NQC - 3)
                extras = []
                if pending:
                    extras += tail_items(*pending.pop(0))
                if b1_items:
                    take, b1_items = b1_items[:per_qc], b1_items[per_qc:]
                    extras += take
                weave(slots, extras)
                attn_fin(0, qc, ctx)
                pending.append((0, qc))

            # Phase C: batch-1 attention.
            for qc in range(NQC):
                sc, av, ctx = attn_parts(1, qc)
                extras = []
                if pending:
                    extras += tail_items(*pending.pop(0))
                weave(qc_slots(sc, av), extras)
                attn_fin(1, qc, ctx)
                pending.append((1, qc))

            # Final drain: ScalarE is idle now; alternate copy engines and
            # let ps_shared double-buffer the pso tiles.
            while pending:
                for it in tail_items(*pending.pop(0),
                                     engines=("scalar", "vector")):
                    it()

    nc.compile()
    return nc


def shard_inputs(x, Wq, bq, Wk, bk, Wv, bv, Wo, bo, S=S_FULL):
    """Host-side sharding: returns list of 8 per-core input dicts."""
    x = np.asarray(x, dtype=np.float32)
    xT = np.ascontiguousarray(x.transpose(0, 2, 1)).astype(np.float16)  # [B, D, S]
    in_maps = []
    for c in range(NCORES):
        sl = slice(c * DC, (c + 1) * DC)
        in_maps.append({
            "xT": xT,
            "wqT": np.ascontiguousarray(np.asarray(Wq)[sl, :].T).astype(np.float16),
            "wkT": np.ascontiguousarray(np.asarray(Wk)[sl, :].T).astype(np.float16),
            "wvT": np.ascontiguousarray(np.asarray(Wv)[sl, :].T).astype(np.float16),
            "woT": np.ascontiguousarray(np.asarray(Wo)[:, sl].T).astype(np.float16),
            "bq": np.ascontiguousarray(np.asarray(bq)[sl], dtype=np.float32),
        })
    return in_maps


_NC_CACHE = {}


def _get_nc(S=S_FULL):
    if S not in _NC_CACHE:
        _NC_CACHE[S] = build_kernel(S)
    return _NC_CACHE[S]


def kernel(x, Wq, bq, Wk, bk, Wv, bv, Wo, bo, _trace=False, _trace_cores=None):
    from concourse import bass_utils

    nc = _get_nc(S_FULL)
    in_maps = shard_inputs(x, Wq, bq, Wk, bk, Wv, bv, Wo, bo)
    kwargs = {}
    if _trace:
        kwargs = dict(trace=True, trace_cores=_trace_cores or [0])
    res = bass_utils.run_bass_kernel_spmd(
        nc, in_maps, core_ids=list(range(NCORES)), **kwargs)
    out = np.zeros((B, S_FULL, D), dtype=np.float32)
    for c in range(NCORES):
        out += res.results[c]["part"]
    # bv is folded out of the device kernel: softmax rows sum to one, so its
    # contribution to the output is the constant Wo @ bv. Add it with bo here.
    bias = (np.asarray(Wo, dtype=np.float64) @ np.asarray(bv, dtype=np.float64)
            + np.asarray(bo, dtype=np.float64))
    out += bias.astype(np.float32)[None, None, :]
    if _trace:
        kernel._last_results = res
    return out


# revision 22
# speedup vs baseline: 1.0893x; 1.0175x over previous
"""Trainium2 Bass kernel for nn_MemoryEfficientAttention (full MHA).

Reference computation (fp32):
    q = split_heads(x @ Wq.T + bq); k, v likewise
    attn = softmax(q @ k.T / sqrt(64))
    out = merge_heads(attn @ v) @ Wo.T + bo

Shapes: B=2, S=4096, D=1024, H=16, head_dim=64.

Sharding across 8 NeuronCores (Megatron attention sharding):
  - 2 heads per core (= 128 of the 1024 projection dims, contiguous slice).
  - Q/K/V projections column-parallel, output projection row-parallel;
    the 8 per-core partial outputs are summed on the host (+ bo).
  - bv never enters the device: softmax rows sum to 1, so its entire effect
    on the output is the constant vector Wo @ bv, added on the host.

Per-core kernel (flash-attention style, nothing S^2-sized touches HBM):
  The kernel is ScalarE-bound: the exp of the S^2 score matrix costs
  ~1.11us per (q-chunk, key-tile) step and 512 such steps = 569us.  All
  emission is therefore woven so ScalarE never starves: projection
  matmul chains are split in half and spread between attention j-steps
  (one chain-half per few steps), never as multi-us blocks.  xt is
  4-deep buffered so input DMA is never gated by tile release.
  Per j-step: scoresT for both heads run concurrently on row-split PE
  tiles, exp on ScalarE (no max subtraction -- scores are bounded), fp16
  PV matmul with an interleaved ones-column accumulating the softmax
  denominator.  Raw output + reciprocal of the denominator leave PSUM
  immediately; normalization resolves during the next chunk.  Output
  projection for chunk qc is woven into chunk qc+1's steps.
"""

import sys

if "/opt/trn_rl_repo" not in sys.path:
    sys.path.insert(0, "/opt/trn_rl_repo")

import numpy as np

B = 2
S_FULL = 4096
D = 1024
H = 16
HD = 64
NCORES = 8
DC = 128          # head dims per core (2 heads x 64)
SCALE = 1.0 / 8.0  # 1/sqrt(64)


def build_kernel(S=S_FULL):
    """Build the per-core Bass program. Returns the compiled Bacc object."""
    import concourse.bacc as bacc
    import concourse.tile as tile
    from concourse import mybir

    f32 = mybir.dt.float32
    f16 = mybir.dt.float16
    AF = mybir.ActivationFunctionType

    KT = D // 128       # k-tiles over the projection contraction dim
    SQ = 512            # q-chunk size
    NQC = S // SQ       # q chunks per batch
    NKT = S // 128      # key tiles per batch
    NM = S // 512       # x chunks for projections

    nc = bacc.Bacc("TRN2", target_bir_lowering=False, debug=False,
                   num_devices=NCORES)

    xT = nc.dram_tensor("xT", [B, D, S], f16, kind="ExternalInput").ap()
    wqT = nc.dram_tensor("wqT", [D, DC], f16, kind="ExternalInput").ap()
    wkT = nc.dram_tensor("wkT", [D, DC], f16, kind="ExternalInput").ap()
    wvT = nc.dram_tensor("wvT", [D, DC], f16, kind="ExternalInput").ap()
    woT = nc.dram_tensor("woT", [DC, D], f16, kind="ExternalInput").ap()
    bq = nc.dram_tensor("bq", [DC], f32, kind="ExternalInput").ap()
    part = nc.dram_tensor("part", [B, S, D], f32, kind="ExternalOutput").ap()

    with tile.TileContext(nc) as tc:
        with (
            tc.tile_pool(name="consts", bufs=1) as consts,
            tc.tile_pool(name="xt", bufs=4) as xt_pool,
            tc.tile_pool(name="qkv", bufs=2) as qkv_pool,
            tc.tile_pool(name="exp", bufs=4) as exp_pool,
            tc.tile_pool(name="exp_pf", bufs=8) as exp_pf_pool,
            tc.tile_pool(name="att", bufs=2) as att_pool,
            tc.tile_pool(name="small", bufs=4) as small_pool,
            tc.tile_pool(name="bounce", bufs=2, space="DRAM") as dram_pool,
            tc.tile_pool(name="ps_scores", bufs=2, space="PSUM") as ps_scores,
            tc.tile_pool(name="ps_acc", bufs=2, space="PSUM") as ps_acc,
            tc.tile_pool(name="ps_shared", bufs=2, space="PSUM") as ps_shared,
        ):
            # ---- constants ----
            wq_sb = consts.tile([128, KT, DC], f16)
            wk_sb = consts.tile([128, KT, DC], f16)
            wv_sb = consts.tile([128, KT, DC], f16)
            wo_sb = consts.tile([128, D], f16)
            bq_sb = consts.tile([128, 1], f32)

            # tiny dummy exp so the ACT table set loads during startup DMAs
            warm = consts.tile([128, 1], f32)
            nc.vector.memset(warm[:], 0.0)
            nc.scalar.activation(warm[:], warm[:], AF.Exp, scale=1.0)

            # k/q weights + bias first (first proj chains need them);
            # wv next, wo last (only needed ~30us in).  bk is dropped
            # entirely: it shifts every score by (q.bk)/8, constant along
            # the key axis, which cancels in the softmax.
            nc.sync.dma_start(
                out=wk_sb[:], in_=wkT.rearrange("(kt p) m -> p kt m", p=128))
            nc.sync.dma_start(
                out=wq_sb[:], in_=wqT.rearrange("(kt p) m -> p kt m", p=128))
            nc.sync.dma_start(
                out=bq_sb[:], in_=bq.rearrange("(p o) -> p o", o=1))
            nc.gpsimd.dma_start(
                out=wv_sb[:], in_=wvT.rearrange("(kt p) m -> p kt m", p=128))
            nc.gpsimd.dma_start(out=wo_sb[:], in_=woT)

            state = [None, None]  # per-batch dict of tiles

            def alloc_batch(b):
                qT_sb = qkv_pool.tile([128, S], f16, tag="qT", name=f"qT_{b}")
                kT_sb = qkv_pool.tile([128, S], f16, tag="kT", name=f"kT_{b}")
                # v natural layout, per key-tile: [vA(64) | 1 | vB(64) | 1]
                v_sb = qkv_pool.tile([128, NKT, 130], f16, tag="v",
                                     name=f"v_{b}")
                nc.vector.memset(v_sb[:, :, 64:65], 1.0)
                nc.vector.memset(v_sb[:, :, 129:130], 1.0)
                attT_sb = att_pool.tile([128, S], f16, tag="attT",
                                        name=f"attT_{b}")
                den_dram = dram_pool.tile([2, S], f16, tag="den",
                                          name=f"den_{b}")
                state[b] = dict(qT=qT_sb, kT=kT_sb, v=v_sb, attT=attT_sb,
                                den=den_dram)

            def proj_dma(b, m):
                """Allocate the x tile for chunk m and return its DMA item.
                Call order must match usage order (per-tag ring)."""
                xt = xt_pool.tile([128, KT, 512], f16, tag="xt",
                                  name=f"xt_{b}_{m}")

                def run():
                    xsrc = xT[b][:, m * 512:(m + 1) * 512].rearrange(
                        "(kt p) s -> p kt s", p=128)
                    half = KT // 2
                    nc.sync.dma_start(out=xt[:, 0:half, :],
                                      in_=xsrc[:, 0:half, :])
                    nc.gpsimd.dma_start(out=xt[:, half:, :],
                                        in_=xsrc[:, half:, :])
                return xt, run

            def proj_chains(b, m, xt):
                """Projection matmul chains for chunk m as 8 small items
                (each <=1us of PE). PSUM tiles alloc at emission time so
                the shared-ring order matches usage order."""
                st = state[b]
                items = []

                def qk_chain(w_sb, b_sb, dst):
                    cell = {}

                    def half1():
                        cell["ps"] = ps_shared.tile(
                            [128, 512], f32, tag="shared",
                            name=f"ps_{b}_{m}_{id(w_sb)}")
                        for j in range(KT // 2):
                            nc.tensor.matmul(
                                cell["ps"][:], lhsT=w_sb[:, j, :],
                                rhs=xt[:, j, :],
                                start=(j == 0), stop=False)

                    def half2():
                        ps = cell["ps"]
                        for j in range(KT // 2, KT):
                            nc.tensor.matmul(
                                ps[:], lhsT=w_sb[:, j, :], rhs=xt[:, j, :],
                                start=False, stop=(j == KT - 1))
                        dslice = dst[:, m * 512:(m + 1) * 512]
                        if b_sb is None:
                            nc.vector.tensor_copy(dslice, ps[:])
                        else:
                            nc.vector.tensor_scalar_add(dslice, ps[:], b_sb[:])
                    return [half1, half2]

                items += qk_chain(wk_sb, None, st["kT"])
                items += qk_chain(wq_sb, bq_sb, st["qT"])

                # V in natural layout: x-tile stationary, Wv moving.
                # Split in halves like q/k so no woven item exceeds ~1us.
                def v_items(t):
                    cell = {}

                    def half1():
                        cell["ps"] = ps_shared.tile(
                            [128, 512], f32, tag="shared",
                            name=f"psv_{b}_{m}_{t}")
                        for j in range(KT // 2):
                            nc.tensor.matmul(
                                cell["ps"][:, 0:DC],
                                lhsT=xt[:, j, t * 128:(t + 1) * 128],
                                rhs=wv_sb[:, j, :],
                                start=(j == 0), stop=False)

                    def half2():
                        psv = cell["ps"]
                        for j in range(KT // 2, KT):
                            nc.tensor.matmul(
                                psv[:, 0:DC],
                                lhsT=xt[:, j, t * 128:(t + 1) * 128],
                                rhs=wv_sb[:, j, :],
                                start=False, stop=(j == KT - 1))
                        kt_idx = m * 4 + t
                        nc.vector.tensor_copy(
                            st["v"][:, kt_idx, 0:64], psv[:, 0:64])
                        nc.vector.tensor_copy(
                            st["v"][:, kt_idx, 65:129], psv[:, 64:128])
                    return [half1, half2]

                for t in range(4):
                    items += v_items(t)
                return items

            def attn_parts(b, qc, pf_tag=None):
                """One q-chunk of 512 rows, split into scores+exp parts and
                AV parts so emission can skew AV behind scores (the first
                AV of a chunk waits on the acc-bank release; skewing keeps
                it from blocking ready scores in the in-order PE queue).
                pf_tag: use a dedicated deep ex ring (phase-A prefetch)."""
                st = state[b]
                qT_sb, kT_sb, v_sb = st["qT"], st["kT"], st["v"]
                q0, q1 = qc * SQ, (qc + 1) * SQ
                ctx = {"ex": {}}

                def sc_part(j):
                    def run():
                        k0, k1 = j * 128, (j + 1) * 128
                        pss = ps_scores.tile([128, 2 * SQ], f32, tag="scores",
                                             name=f"pss_{b}_{qc}_{j}")
                        for hh in range(2):
                            nc.tensor.matmul(
                                pss[:, hh * SQ:(hh + 1) * SQ],
                                lhsT=kT_sb[hh * 64:(hh + 1) * 64, k0:k1],
                                rhs=qT_sb[hh * 64:(hh + 1) * 64, q0:q1],
                                start=True, stop=True)
                        pool = exp_pf_pool if (pf_tag and j < 8) else exp_pool
                        ex = pool.tile([128, 2 * SQ], f16,
                                       tag="exp_pf" if (pf_tag and j < 8)
                                       else "exp",
                                       name=f"ex_{b}_{qc}_{j}")
                        nc.scalar.activation(ex[:], pss[:], AF.Exp,
                                             scale=SCALE)
                        ctx["ex"][j] = ex
                    return run

                def av_part(j):
                    def run():
                        if j == 0:
                            ctx["acc"] = [
                                ps_acc.tile([128, SQ], f32, tag="acc",
                                            name=f"acc{hh}_{b}_{qc}")
                                for hh in range(2)]
                        accs = ctx["acc"]
                        ex = ctx["ex"].pop(j)
                        for hh in range(2):
                            nc.tensor.matmul(
                                accs[hh][0:65, :],
                                lhsT=v_sb[:, j, hh * 65:(hh + 1) * 65],
                                rhs=ex[:, hh * SQ:(hh + 1) * SQ],
                                start=(j == 0), stop=(j == NKT - 1))
                    return run

                return ([sc_part(j) for j in range(NKT)],
                        [av_part(j) for j in range(NKT)], ctx)

            def attn_fin(b, qc, ctx, use_scalar=False):
                """PSUM release + softmax normalization for a q-chunk.
                use_scalar: offload one head's release copies to ScalarE
                (only safe after the last exp has been emitted)."""
                st = state[b]
                attT_sb, den_dram = st["attT"], st["den"]
                q0, q1 = qc * SQ, (qc + 1) * SQ
                accs = ctx["acc"]
                for hh in range(2):
                    if use_scalar and hh == 1:
                        nc.scalar.copy(
                            attT_sb[hh * 64:(hh + 1) * 64, q0:q1],
                            accs[hh][0:64, :])
                    else:
                        nc.vector.tensor_copy(
                            attT_sb[hh * 64:(hh + 1) * 64, q0:q1],
                            accs[hh][0:64, :])
                    dn = small_pool.tile([1, SQ], f32, tag="dn",
                                         name=f"dn_{b}_{qc}_{hh}")
                    nc.vector.tensor_copy(dn[:], accs[hh][64:65, :])
                    rcp = small_pool.tile([1, SQ], f32, tag="rcp",
                                          name=f"rcp_{b}_{qc}_{hh}")
                    nc.vector.reciprocal_approx_fast(rcp[:], dn[:])
                    rcp16 = small_pool.tile([1, SQ], f16, tag="rcp16",
                                            name=f"rcp16_{b}_{qc}_{hh}")
                    with nc.allow_low_precision(reason="fp16 softmax recip"):
                        nc.vector.tensor_copy(rcp16[:], rcp[:])
                    nc.gpsimd.dma_start(out=den_dram[hh, q0:q1], in_=rcp16[:])

                # broadcast the reciprocal + normalize; resolves during the
                # next chunk's attention
                for hh in range(2):
                    bc = small_pool.tile([128, SQ], f16, tag="bcast",
                                         name=f"bc_{b}_{qc}_{hh}")
                    bch = bc[hh * 64:(hh + 1) * 64, :]
                    rd = den_dram[hh, q0:q1]
                    bcast_src = rd.__class__(
                        tensor=rd.tensor, offset=rd.offset,
                        ap=[[0, 64]] + list(rd.ap),
                    )
                    nc.gpsimd.dma_start(out=bch, in_=bcast_src)
                    nc.vector.tensor_mul(
                        attT_sb[hh * 64:(hh + 1) * 64, q0:q1],
                        attT_sb[hh * 64:(hh + 1) * 64, q0:q1],
                        bch,
                    )

            def tail_items(b, qc, engines=("vector",)):
                """Output projection for a q-chunk as 8 single-tile items,
                emitted one chunk late so the normalization chain has
                resolved and the in-order PE never stalls on it."""
                st = state[b]
                attT_sb = st["attT"]
                q0 = qc * SQ
                items = []
                for sti in range(SQ // 128):
                    for oc in range(D // 512):
                        def run(sti=sti, oc=oc, n=sti * 2 + oc):
                            s0 = q0 + sti * 128
                            s1 = s0 + 128
                            pso = ps_shared.tile(
                                [128, 512], f32, tag="shared",
                                name=f"pso_{b}_{qc}_{sti}_{oc}")
                            nc.tensor.matmul(
                                pso[:], lhsT=attT_sb[:, s0:s1],
                                rhs=wo_sb[:, oc * 512:(oc + 1) * 512],
                                start=True, stop=True)
                            ob = small_pool.tile([128, 512], f32, tag="ob",
                                                 name=f"ob_{b}_{qc}_{sti}_{oc}")
                            eng = engines[n % len(engines)]
                            if eng == "scalar":
                                nc.scalar.copy(ob[:], pso[:])
                            else:
                                nc.vector.tensor_copy(ob[:], pso[:])
                            nc.sync.dma_start(
                                out=part[b, s0:s1, oc * 512:(oc + 1) * 512],
                                in_=ob[:])
                        items.append(run)
                return items

            def weave(steps, extras):
                """Run all steps in order, spreading extras evenly between
                them (extras trail: first extra after the first step)."""
                n, e = len(steps), len(extras)
                ei = 0
                for i, s in enumerate(steps):
                    s()
                    while ei < e and (ei + 1) * n <= (i + 1) * e:
                        extras[ei]()
                        ei += 1
                while ei < e:
                    extras[ei]()
                    ei += 1

            def mk_slot(*fns):
                def run():
                    for f in fns:
                        f()
                return run

            def window(sc, av, carry):
                """Stitched slot window for a standard chunk: carry (the
                previous chunk's fin) lands at slot 0, av lags sc by 2,
                av30/av31 run in a final sc-free slot.  Returns (slots,
                carry_out_prefix) where the caller appends fin."""
                slots = [mk_slot(sc[0], *carry), mk_slot(sc[1])]
                for j in range(2, NKT):
                    slots.append(mk_slot(sc[j], av[j - 2]))
                slots.append(mk_slot(av[NKT - 2], av[NKT - 1]))
                return slots

            # ---- emission schedule ----
            # PE warmup: ~16 dummy matmuls ramp the PE out of its low
            # p-state (0.65GHz cold, 2.4GHz after ~3us of execution) while
            # the startup DMAs are in flight, so the first projection
            # chains run at speed.
            dummy = consts.tile([128, 256], f16)
            nc.vector.memset(dummy[:], 0.0)
            for w in range(12):
                psw = ps_shared.tile([128, 512], f32, tag="shared",
                                     name=f"warmmm_{w}")
                nc.tensor.matmul(psw[:, 0:256], lhsT=dummy[:, 0:128],
                                 rhs=dummy[:], start=True, stop=True)

            # Phase A: batch-0 chunk-0 k/q chains up front, then chunks
            # 1..7 woven with the first q-chunk's slots (4 per chunk, one
            # chunk-DMA ahead).  qc1's first PF score+exp parts are
            # prefetched into phase A (deep ex ring) so ScalarE has more
            # than one chunk's exp work while PE grinds projections.
            PF = 8
            alloc_batch(0)
            xts = {}
            for m in range(2):
                xts[m], dma = proj_dma(0, m)
                dma()
            ch0 = proj_chains(0, 0, xts[0])
            for it in ch0[:4]:
                it()
            sc0, av0, ctx0 = attn_parts(0, 0)
            sc1, av1, ctx1 = attn_parts(0, 1, pf_tag="exp_pf")
            pf_sched = {2: [0, 1], 3: [2, 3], 4: [4], 5: [5], 6: [6], 7: [7]}

            def slot_a(j):
                def run():
                    sc0[j]()
                    if j >= 2:
                        av0[j - 2]()
                return run

            for m in range(1, NM):
                extras = []
                if m == 1:
                    extras += ch0[4:]
                if m + 1 < NM:
                    xts[m + 1], dma = proj_dma(0, m + 1)
                    extras.append(dma)
                extras += proj_chains(0, m, xts[m])
                extras += [sc1[p] for p in pf_sched.get(m, [])]
                weave([slot_a(j) for j in range(4 * (m - 1), 4 * m)], extras)
            for j in range(4 * (NM - 1), NKT):
                slot_a(j)()
            mk_slot(av0[NKT - 2], av0[NKT - 1])()
            carry = [lambda: attn_fin(0, 0, ctx0)]
            pending = [(0, 0)]

            # Phase B: batch-0 qc1 first catches up the prefetched AVs
            # (its sc j>=8 allocations require av[j-4] emitted first),
            # then qc 2..7 with batch-1 projections woven over qc 2..6
            # and tails (2 chunks behind) woven throughout.
            def qc1_slots(carry):
                slots = [mk_slot(sc1[8], *carry), mk_slot(sc1[9]),
                         mk_slot(sc1[10], av1[0], av1[1]),
                         mk_slot(sc1[11], av1[2], av1[3]),
                         mk_slot(av1[4], av1[5]),
                         mk_slot(av1[6], av1[7], av1[8]),
                         mk_slot(sc1[12], av1[9]),
                         mk_slot(sc1[13], av1[10]),
                         mk_slot(sc1[14], av1[11], av1[12]),
                         mk_slot(sc1[15], av1[13])]
                for j in range(16, NKT):
                    slots.append(mk_slot(sc1[j], av1[j - 2]))
                slots.append(mk_slot(av1[NKT - 2], av1[NKT - 1]))
                return slots

            TAIL_DELAY = 2
            b1_items = []
            for qc in range(1, NQC):
                if qc == 1:
                    slots = qc1_slots(carry)
                    ctx = ctx1
                else:
                    sc, av, ctx = attn_parts(0, qc)
                    slots = window(sc, av, carry)
                if qc == 2:
                    alloc_batch(1)
                    b1_xts = {}
                    b1_xts[0], dma0 = proj_dma(1, 0)
                    b1_items.append(dma0)
                    for m in range(NM):
                        if m + 1 < NM:
                            b1_xts[m + 1], dma = proj_dma(1, m + 1)
                            b1_items.append(dma)
                        b1_items += proj_chains(1, m, b1_xts[m])
                    per_qc = (len(b1_items) + NQC - 4) // (NQC - 3)
                extras = []
                if len(pending) >= TAIL_DELAY:
                    extras += tail_items(*pending.pop(0))
                if b1_items:
                    take, b1_items = b1_items[:per_qc], b1_items[per_qc:]
                    extras += take
                weave(slots, extras)
                carry = [lambda ctx=ctx, qc=qc: attn_fin(0, qc, ctx)]
                pending.append((0, qc))

            # Phase C: batch-1 attention.
            for qc in range(NQC):
                sc, av, ctx = attn_parts(1, qc)
                extras = []
                if len(pending) >= TAIL_DELAY:
                    extras += tail_items(*pending.pop(0))
                weave(window(sc, av, carry), extras)
                last = (qc == NQC - 1)
                carry = [lambda ctx=ctx, qc=qc, last=last: attn_fin(
                    1, qc, ctx, use_scalar=last)]
                pending.append((1, qc))

            # Final drain: the last fin runs under the second-to-last
            # chunk's output projection; ScalarE helps with copies and
            # ps_shared double-buffers the pso tiles.
            for f in carry:
                f()
            while pending:
                for it in tail_items(*pending.pop(0),
                                     engines=("scalar", "vector")):
                    it()

    nc.compile()
    return nc


def shard_inputs(x, Wq, bq, Wk, bk, Wv, bv, Wo, bo, S=S_FULL):
    """Host-side sharding: returns list of 8 per-core input dicts."""
    x = np.asarray(x, dtype=np.float32)
    xT = np.ascontiguousarray(x.transpose(0, 2, 1)).astype(np.float16)  # [B, D, S]
    in_maps = []
    for c in range(NCORES):
        sl = slice(c * DC, (c + 1) * DC)
        in_maps.append({
            "xT": xT,
            "wqT": np.ascontiguousarray(np.asarray(Wq)[sl, :].T).astype(np.float16),
            "wkT": np.ascontiguousarray(np.asarray(Wk)[sl, :].T).astype(np.float16),
            "wvT": np.ascontiguousarray(np.asarray(Wv)[sl, :].T).astype(np.float16),
            "woT": np.ascontiguousarray(np.asarray(Wo)[:, sl].T).astype(np.float16),
            "bq": np.ascontiguousarray(np.asarray(bq)[sl], dtype=np.float32),
        })
    return in_maps


_NC_CACHE = {}


def _get_nc(S=S_FULL):
    if S not in _NC_CACHE:
        _NC_CACHE[S] = build_kernel(S)
    return _NC_CACHE[S]


def kernel(x, Wq, bq, Wk, bk, Wv, bv, Wo, bo, _trace=False, _trace_cores=None):
    from concourse import bass_utils

    nc = _get_nc(S_FULL)
    in_maps = shard_inputs(x, Wq, bq, Wk, bk, Wv, bv, Wo, bo)
    kwargs = {}
    if _trace:
        kwargs = dict(trace=True, trace_cores=_trace_cores or [0])
    res = bass_utils.run_bass_kernel_spmd(
        nc, in_maps, core_ids=list(range(NCORES)), **kwargs)
    out = np.zeros((B, S_FULL, D), dtype=np.float32)
    for c in range(NCORES):
        out += res.results[c]["part"]
    # bv is folded out of the device kernel: softmax rows sum to one, so its
    # contribution to the output is the constant Wo @ bv. Add it with bo here.
    bias = (np.asarray(Wo, dtype=np.float64) @ np.asarray(bv, dtype=np.float64)
            + np.asarray(bo, dtype=np.float64))
    out += bias.astype(np.float32)[None, None, :]
    if _trace:
        kernel._last_results = res
    return out


# revision 29
# speedup vs baseline: 1.1108x; 1.0198x over previous
"""Trainium2 Bass kernel for nn_MemoryEfficientAttention (full MHA).

Reference computation (fp32):
    q = split_heads(x @ Wq.T + bq); k, v likewise
    attn = softmax(q @ k.T / sqrt(64))
    out = merge_heads(attn @ v) @ Wo.T + bo

Shapes: B=2, S=4096, D=1024, H=16, head_dim=64.

Sharding across 8 NeuronCores (Megatron attention sharding):
  - 2 heads per core (= 128 of the 1024 projection dims, contiguous slice).
  - Q/K/V projections column-parallel, output projection row-parallel;
    the 8 per-core partial outputs are summed on the host (+ bo).
  - bv never enters the device: softmax rows sum to 1, so its entire effect
    on the output is the constant vector Wo @ bv, added on the host.

Per-core kernel (flash-attention style, nothing S^2-sized touches HBM):
  The kernel is ScalarE-bound: the exp of the S^2 score matrix costs
  ~1.11us per (q-chunk, key-tile) step and 512 such steps = 569us.  All
  emission is therefore woven so ScalarE never starves: projection
  matmul chains are split in half and spread between attention j-steps
  (one chain-half per few steps), never as multi-us blocks.  xt is
  4-deep buffered so input DMA is never gated by tile release.
  Per j-step: scoresT for both heads run concurrently on row-split PE
  tiles, exp on ScalarE (no max subtraction -- scores are bounded), fp16
  PV matmul with an interleaved ones-column accumulating the softmax
  denominator.  Raw output + reciprocal of the denominator leave PSUM
  immediately; normalization resolves during the next chunk.  Output
  projection for chunk qc is woven into chunk qc+1's steps.
"""

import sys

if "/opt/trn_rl_repo" not in sys.path:
    sys.path.insert(0, "/opt/trn_rl_repo")

import numpy as np

B = 2
S_FULL = 4096
D = 1024
H = 16
HD = 64
NCORES = 8
DC = 128          # head dims per core (2 heads x 64)
SCALE = 1.0 / 8.0  # 1/sqrt(64)


def build_kernel(S=S_FULL):
    """Build the per-core Bass program. Returns the compiled Bacc object."""
    import concourse.bacc as bacc
    import concourse.tile as tile
    from concourse import mybir

    f32 = mybir.dt.float32
    f16 = mybir.dt.float16
    AF = mybir.ActivationFunctionType

    KT = D // 128       # k-tiles over the projection contraction dim
    SQ = 512            # q-chunk size
    NQC = S // SQ       # q chunks per batch
    NKT = S // 128      # key tiles per batch
    NM = S // 512       # x chunks for projections

    nc = bacc.Bacc("TRN2", target_bir_lowering=False, debug=False,
                   num_devices=NCORES)

    xT = nc.dram_tensor("xT", [B, D, S], f16, kind="ExternalInput").ap()
    wqT = nc.dram_tensor("wqT", [D, DC], f16, kind="ExternalInput").ap()
    wkT = nc.dram_tensor("wkT", [D, DC], f16, kind="ExternalInput").ap()
    wvT = nc.dram_tensor("wvT", [D, DC], f16, kind="ExternalInput").ap()
    woT = nc.dram_tensor("woT", [DC, D], f16, kind="ExternalInput").ap()
    bq = nc.dram_tensor("bq", [DC], f32, kind="ExternalInput").ap()
    part = nc.dram_tensor("part", [B, S, D], f32, kind="ExternalOutput").ap()

    with tile.TileContext(nc) as tc:
        with (
            tc.tile_pool(name="consts", bufs=1) as consts,
            tc.tile_pool(name="xt", bufs=4) as xt_pool,
            tc.tile_pool(name="qkv", bufs=2) as qkv_pool,
            tc.tile_pool(name="exp", bufs=4) as exp_pool,
            tc.tile_pool(name="exp_pf", bufs=8) as exp_pf_pool,
            tc.tile_pool(name="att", bufs=2) as att_pool,
            tc.tile_pool(name="small", bufs=4) as small_pool,
            tc.tile_pool(name="bounce", bufs=2, space="DRAM") as dram_pool,
            tc.tile_pool(name="ps_scores", bufs=2, space="PSUM") as ps_scores,
            tc.tile_pool(name="ps_acc", bufs=2, space="PSUM") as ps_acc,
            tc.tile_pool(name="ps_shared", bufs=2, space="PSUM") as ps_shared,
        ):
            # ---- constants ----
            wq_sb = consts.tile([128, KT, DC], f16)
            wk_sb = consts.tile([128, KT, DC], f16)
            wv_sb = consts.tile([128, KT, DC], f16)
            wo_sb = consts.tile([128, D], f16)
            bq_sb = consts.tile([128, 1], f32)

            # tiny dummy exp so the ACT table set loads during startup DMAs
            warm = consts.tile([128, 1], f32)
            nc.vector.memset(warm[:], 0.0)
            nc.scalar.activation(warm[:], warm[:], AF.Exp, scale=1.0)

            # DMA order is latency-critical: the first k-chain needs wk +
            # both xt halves, so wk leads the sync queue and the xt DMAs
            # (emitted right after, in phase A below) go next on both
            # queues; wq/bq follow on sync, wv/wo trail on gpsimd.  bk is
            # dropped entirely: it shifts every score by (q.bk)/8,
            # constant along the key axis, which cancels in the softmax.
            def load_consts_head():
                nc.sync.dma_start(
                    out=wk_sb[:],
                    in_=wkT.rearrange("(kt p) m -> p kt m", p=128))

            def load_consts_tail():
                nc.sync.dma_start(
                    out=wq_sb[:],
                    in_=wqT.rearrange("(kt p) m -> p kt m", p=128))
                nc.sync.dma_start(
                    out=bq_sb[:], in_=bq.rearrange("(p o) -> p o", o=1))
                nc.gpsimd.dma_start(
                    out=wv_sb[:],
                    in_=wvT.rearrange("(kt p) m -> p kt m", p=128))
                nc.gpsimd.dma_start(out=wo_sb[:], in_=woT)

            state = [None, None]  # per-batch dict of tiles

            def alloc_batch(b):
                qT_sb = qkv_pool.tile([128, S], f16, tag="qT", name=f"qT_{b}")
                kT_sb = qkv_pool.tile([128, S], f16, tag="kT", name=f"kT_{b}")
                # v natural layout, per key-tile: [vA(64) | 1 | vB(64) | 1]
                v_sb = qkv_pool.tile([128, NKT, 130], f16, tag="v",
                                     name=f"v_{b}")
                nc.vector.memset(v_sb[:, :, 64:65], 1.0)
                nc.vector.memset(v_sb[:, :, 129:130], 1.0)
                attT_sb = att_pool.tile([128, S], f16, tag="attT",
                                        name=f"attT_{b}")
                den_dram = dram_pool.tile([2, S], f16, tag="den",
                                          name=f"den_{b}")
                state[b] = dict(qT=qT_sb, kT=kT_sb, v=v_sb, attT=attT_sb,
                                den=den_dram)

            def proj_dma(b, m):
                """Allocate the x tile for chunk m and return its DMA item.
                Call order must match usage order (per-tag ring)."""
                xt = xt_pool.tile([128, KT, 512], f16, tag="xt",
                                  name=f"xt_{b}_{m}")

                def run():
                    xsrc = xT[b][:, m * 512:(m + 1) * 512].rearrange(
                        "(kt p) s -> p kt s", p=128)
                    half = KT // 2
                    nc.sync.dma_start(out=xt[:, 0:half, :],
                                      in_=xsrc[:, 0:half, :])
                    nc.gpsimd.dma_start(out=xt[:, half:, :],
                                        in_=xsrc[:, half:, :])
                return xt, run

            def proj_chains(b, m, xt):
                """Projection matmul chains for chunk m as 8 small items
                (each <=1us of PE). PSUM tiles alloc at emission time so
                the shared-ring order matches usage order."""
                st = state[b]
                items = []

                def qk_chain(w_sb, b_sb, dst):
                    cell = {}

                    def half1():
                        cell["ps"] = ps_shared.tile(
                            [128, 512], f32, tag="shared",
                            name=f"ps_{b}_{m}_{id(w_sb)}")
                        for j in range(KT // 2):
                            nc.tensor.matmul(
                                cell["ps"][:], lhsT=w_sb[:, j, :],
                                rhs=xt[:, j, :],
                                start=(j == 0), stop=False)

                    def half2():
                        ps = cell["ps"]
                        for j in range(KT // 2, KT):
                            nc.tensor.matmul(
                                ps[:], lhsT=w_sb[:, j, :], rhs=xt[:, j, :],
                                start=False, stop=(j == KT - 1))
                        dslice = dst[:, m * 512:(m + 1) * 512]
                        if b_sb is None:
                            nc.vector.tensor_copy(dslice, ps[:])
                        else:
                            nc.vector.tensor_scalar_add(dslice, ps[:], b_sb[:])
                    return [half1, half2]

                items += qk_chain(wk_sb, None, st["kT"])
                items += qk_chain(wq_sb, bq_sb, st["qT"])

                # V in natural layout: x-tile stationary, Wv moving.
                # Split in halves like q/k so no woven item exceeds ~1us.
                def v_items(t):
                    cell = {}

                    def half1():
                        cell["ps"] = ps_shared.tile(
                            [128, 512], f32, tag="shared",
                            name=f"psv_{b}_{m}_{t}")
                        for j in range(KT // 2):
                            nc.tensor.matmul(
                                cell["ps"][:, 0:DC],
                                lhsT=xt[:, j, t * 128:(t + 1) * 128],
                                rhs=wv_sb[:, j, :],
                                start=(j == 0), stop=False)

                    def half2():
                        psv = cell["ps"]
                        for j in range(KT // 2, KT):
                            nc.tensor.matmul(
                                psv[:, 0:DC],
                                lhsT=xt[:, j, t * 128:(t + 1) * 128],
                                rhs=wv_sb[:, j, :],
                                start=False, stop=(j == KT - 1))
                        kt_idx = m * 4 + t
                        nc.vector.tensor_copy(
                            st["v"][:, kt_idx, 0:64], psv[:, 0:64])
                        nc.vector.tensor_copy(
                            st["v"][:, kt_idx, 65:129], psv[:, 64:128])
                    return [half1, half2]

                for t in range(4):
                    items += v_items(t)
                return items

            def attn_parts(b, qc, pf_tag=None):
                """One q-chunk of 512 rows, split into scores+exp parts and
                AV parts so emission can skew AV behind scores (the first
                AV of a chunk waits on the acc-bank release; skewing keeps
                it from blocking ready scores in the in-order PE queue).
                pf_tag: use a dedicated deep ex ring (phase-A prefetch)."""
                st = state[b]
                qT_sb, kT_sb, v_sb = st["qT"], st["kT"], st["v"]
                q0, q1 = qc * SQ, (qc + 1) * SQ
                ctx = {"ex": {}}

                def sc_part(j):
                    def run():
                        k0, k1 = j * 128, (j + 1) * 128
                        pss = ps_scores.tile([128, 2 * SQ], f32, tag="scores",
                                             name=f"pss_{b}_{qc}_{j}")
                        for hh in range(2):
                            nc.tensor.matmul(
                                pss[:, hh * SQ:(hh + 1) * SQ],
                                lhsT=kT_sb[hh * 64:(hh + 1) * 64, k0:k1],
                                rhs=qT_sb[hh * 64:(hh + 1) * 64, q0:q1],
                                start=True, stop=True)
                        pool = exp_pf_pool if (pf_tag and j < 8) else exp_pool
                        ex = pool.tile([128, 2 * SQ], f16,
                                       tag="exp_pf" if (pf_tag and j < 8)
                                       else "exp",
                                       name=f"ex_{b}_{qc}_{j}")
                        nc.scalar.activation(ex[:], pss[:], AF.Exp,
                                             scale=SCALE)
                        ctx["ex"][j] = ex
                    return run

                def av_part(j):
                    def run():
                        if j == 0:
                            ctx["acc"] = [
                                ps_acc.tile([128, SQ], f32, tag="acc",
                                            name=f"acc{hh}_{b}_{qc}")
                                for hh in range(2)]
                        accs = ctx["acc"]
                        ex = ctx["ex"].pop(j)
                        for hh in range(2):
                            nc.tensor.matmul(
                                accs[hh][0:65, :],
                                lhsT=v_sb[:, j, hh * 65:(hh + 1) * 65],
                                rhs=ex[:, hh * SQ:(hh + 1) * SQ],
                                start=(j == 0), stop=(j == NKT - 1))
                    return run

                return ([sc_part(j) for j in range(NKT)],
                        [av_part(j) for j in range(NKT)], ctx)

            def attn_fin(b, qc, ctx, use_scalar=False):
                """PSUM release + softmax normalization for a q-chunk.
                use_scalar: offload one head's release copies to ScalarE
                (only safe after the last exp has been emitted)."""
                st = state[b]
                attT_sb, den_dram = st["attT"], st["den"]
                q0, q1 = qc * SQ, (qc + 1) * SQ
                accs = ctx["acc"]
                for hh in range(2):
                    if use_scalar and hh == 1:
                        nc.scalar.copy(
                            attT_sb[hh * 64:(hh + 1) * 64, q0:q1],
                            accs[hh][0:64, :])
                    else:
                        nc.vector.tensor_copy(
                            attT_sb[hh * 64:(hh + 1) * 64, q0:q1],
                            accs[hh][0:64, :])
                    dn = small_pool.tile([1, SQ], f32, tag="dn",
                                         name=f"dn_{b}_{qc}_{hh}")
                    nc.vector.tensor_copy(dn[:], accs[hh][64:65, :])
                    rcp = small_pool.tile([1, SQ], f32, tag="rcp",
                                          name=f"rcp_{b}_{qc}_{hh}")
                    nc.vector.reciprocal_approx_fast(rcp[:], dn[:])
                    rcp16 = small_pool.tile([1, SQ], f16, tag="rcp16",
                                            name=f"rcp16_{b}_{qc}_{hh}")
                    with nc.allow_low_precision(reason="fp16 softmax recip"):
                        nc.vector.tensor_copy(rcp16[:], rcp[:])
                    nc.gpsimd.dma_start(out=den_dram[hh, q0:q1], in_=rcp16[:])

                # broadcast the reciprocal + normalize; resolves during the
                # next chunk's attention
                for hh in range(2):
                    bc = small_pool.tile([128, SQ], f16, tag="bcast",
                                         name=f"bc_{b}_{qc}_{hh}")
                    bch = bc[hh * 64:(hh + 1) * 64, :]
                    rd = den_dram[hh, q0:q1]
                    bcast_src = rd.__class__(
                        tensor=rd.tensor, offset=rd.offset,
                        ap=[[0, 64]] + list(rd.ap),
                    )
                    nc.gpsimd.dma_start(out=bch, in_=bcast_src)
                    nc.vector.tensor_mul(
                        attT_sb[hh * 64:(hh + 1) * 64, q0:q1],
                        attT_sb[hh * 64:(hh + 1) * 64, q0:q1],
                        bch,
                    )

            def tail_items(b, qc, engines=("vector",)):
                """Output projection for a q-chunk as 8 single-tile items,
                emitted one chunk late so the normalization chain has
                resolved and the in-order PE never stalls on it."""
                st = state[b]
                attT_sb = st["attT"]
                q0 = qc * SQ
                items = []
                for sti in range(SQ // 128):
                    for oc in range(D // 512):
                        def run(sti=sti, oc=oc, n=sti * 2 + oc):
                            s0 = q0 + sti * 128
                            s1 = s0 + 128
                            pso = ps_shared.tile(
                                [128, 512], f32, tag="shared",
                                name=f"pso_{b}_{qc}_{sti}_{oc}")
                            nc.tensor.matmul(
                                pso[:], lhsT=attT_sb[:, s0:s1],
                                rhs=wo_sb[:, oc * 512:(oc + 1) * 512],
                                start=True, stop=True)
                            ob = small_pool.tile([128, 512], f32, tag="ob",
                                                 name=f"ob_{b}_{qc}_{sti}_{oc}")
                            eng = engines[n % len(engines)]
                            if eng == "scalar":
                                nc.scalar.copy(ob[:], pso[:])
                            else:
                                nc.vector.tensor_copy(ob[:], pso[:])
                            nc.sync.dma_start(
                                out=part[b, s0:s1, oc * 512:(oc + 1) * 512],
                                in_=ob[:])
                        items.append(run)
                return items

            def weave(steps, extras):
                """Run all steps in order, spreading extras evenly between
                them (extras trail: first extra after the first step)."""
                n, e = len(steps), len(extras)
                ei = 0
                for i, s in enumerate(steps):
                    s()
                    while ei < e and (ei + 1) * n <= (i + 1) * e:
                        extras[ei]()
                        ei += 1
                while ei < e:
                    extras[ei]()
                    ei += 1

            def mk_slot(*fns):
                def run():
                    for f in fns:
                        f()
                return run

            def window(sc, av, carry):
                """Stitched slot window for a standard chunk: carry (the
                previous chunk's fin) lands at slot 0, av lags sc by 2,
                av30/av31 run in a final sc-free slot.  Returns (slots,
                carry_out_prefix) where the caller appends fin."""
                slots = [mk_slot(sc[0], *carry), mk_slot(sc[1])]
                for j in range(2, NKT):
                    slots.append(mk_slot(sc[j], av[j - 2]))
                slots.append(mk_slot(av[NKT - 2], av[NKT - 1]))
                return slots

            # ---- emission schedule ----
            # PE warmup: ~16 dummy matmuls ramp the PE out of its low
            # p-state (0.65GHz cold, 2.4GHz after ~3us of execution) while
            # the startup DMAs are in flight, so the first projection
            # chains run at speed.
            load_consts_head()
            dummy = consts.tile([128, 256], f16)
            nc.vector.memset(dummy[:], 0.0)
            for w in range(18):
                psw = ps_shared.tile([128, 512], f32, tag="shared",
                                     name=f"warmmm_{w}")
                nc.tensor.matmul(psw[:, 0:256], lhsT=dummy[:, 0:128],
                                 rhs=dummy[:], start=True, stop=True)

            # Phase A: batch-0 chunk-0 k/q chains up front, then chunks
            # 1..7 woven with the first q-chunk's slots (4 per chunk, one
            # chunk-DMA ahead).  qc1's first PF score+exp parts are
            # prefetched into phase A (deep ex ring) so ScalarE has more
            # than one chunk's exp work while PE grinds projections.
            PF = 8
            alloc_batch(0)
            xts = {}
            xts[0], dma = proj_dma(0, 0)
            dma()
            load_consts_tail()
            xts[1], dma = proj_dma(0, 1)
            dma()
            ch0 = proj_chains(0, 0, xts[0])
            for it in ch0[:4]:
                it()
            sc0, av0, ctx0 = attn_parts(0, 0)
            sc1, av1, ctx1 = attn_parts(0, 1, pf_tag="exp_pf")
            pf_sched = {2: [0, 1], 3: [2, 3], 4: [4], 5: [5], 6: [6], 7: [7]}

            def slot_a(j):
                def run():
                    sc0[j]()
                    if j >= 2:
                        av0[j - 2]()
                return run

            for m in range(1, NM):
                extras = []
                if m == 1:
                    extras += ch0[4:]
                if m + 1 < NM:
                    xts[m + 1], dma = proj_dma(0, m + 1)
                    extras.append(dma)
                extras += proj_chains(0, m, xts[m])
                extras += [sc1[p] for p in pf_sched.get(m, [])]
                weave([slot_a(j) for j in range(4 * (m - 1), 4 * m)], extras)
            for j in range(4 * (NM - 1), NKT):
                slot_a(j)()
            mk_slot(av0[NKT - 2], av0[NKT - 1])()
            carry = [lambda: attn_fin(0, 0, ctx0)]
            pending = [(0, 0)]

            # Phase B: batch-0 qc1 first catches up the prefetched AVs
            # (its sc j>=8 allocations require av[j-4] emitted first),
            # then qc 2..7 with batch-1 projections woven over qc 2..6
            # and tails (2 chunks behind) woven throughout.
            def qc1_slots(carry):
                slots = [mk_slot(sc1[8], *carry), mk_slot(sc1[9]),
                         mk_slot(sc1[10], av1[0], av1[1]),
                         mk_slot(sc1[11], av1[2], av1[3]),
                         mk_slot(av1[4], av1[5]),
                         mk_slot(av1[6], av1[7], av1[8]),
                         mk_slot(sc1[12], av1[9]),
                         mk_slot(sc1[13], av1[10]),
                         mk_slot(sc1[14], av1[11], av1[12]),
                         mk_slot(sc1[15], av1[13])]
                for j in range(16, NKT):
                    slots.append(mk_slot(sc1[j], av1[j - 2]))
                slots.append(mk_slot(av1[NKT - 2], av1[NKT - 1]))
                return slots

            # Phase B carries no output-projection tails: its PE budget is
            # already full with batch-1 projections (attn 27us + proj 9us
            # vs ScalarE's 35.5us per chunk).  All tails drain in phase C,
            # which has the PE slack (2 per chunk).
            b1_items = []
            for qc in range(1, NQC):
                if qc == 1:
                    slots = qc1_slots(carry)
                    ctx = ctx1
                else:
                    sc, av, ctx = attn_parts(0, qc)
                    slots = window(sc, av, carry)
                if qc == 2:
                    alloc_batch(1)
                    b1_xts = {}
                    b1_xts[0], dma0 = proj_dma(1, 0)
                    b1_items.append(dma0)
                    for m in range(NM):
                        if m + 1 < NM:
                            b1_xts[m + 1], dma = proj_dma(1, m + 1)
                            b1_items.append(dma)
                        b1_items += proj_chains(1, m, b1_xts[m])
                    per_qc = (len(b1_items) + NQC - 4) // (NQC - 3)
                extras = []
                if b1_items:
                    take, b1_items = b1_items[:per_qc], b1_items[per_qc:]
                    extras += take
                weave(slots, extras)
                carry = [lambda ctx=ctx, qc=qc: attn_fin(0, qc, ctx)]
                pending.append((0, qc))

            # Phase C: batch-1 attention, draining two tails per chunk.
            for qc in range(NQC):
                sc, av, ctx = attn_parts(1, qc)
                extras = []
                for _ in range(2):
                    if len(pending) > 2:
                        extras += tail_items(*pending.pop(0))
                weave(window(sc, av, carry), extras)
                last = (qc == NQC - 1)
                carry = [lambda ctx=ctx, qc=qc, last=last: attn_fin(
                    1, qc, ctx, use_scalar=last)]
                pending.append((1, qc))

            # Final drain: the last fin runs under the second-to-last
            # chunk's output projection; ScalarE helps with copies and
            # ps_shared double-buffers the pso tiles.
            for f in carry:
                f()
            while pending:
                for it in tail_items(*pending.pop(0),
                                     engines=("scalar", "vector")):
                    it()

    nc.compile()
    return nc


def shard_inputs(x, Wq, bq, Wk, bk, Wv, bv, Wo, bo, S=S_FULL):
    """Host-side sharding: returns list of 8 per-core input dicts."""
    x = np.asarray(x, dtype=np.float32)
    xT = np.ascontiguousarray(x.transpose(0, 2, 1)).astype(np.float16)  # [B, D, S]
    in_maps = []
    for c in range(NCORES):
        sl = slice(c * DC, (c + 1) * DC)
        in_maps.append({
            "xT": xT,
            "wqT": np.ascontiguousarray(np.asarray(Wq)[sl, :].T).astype(np.float16),
            "wkT": np.ascontiguousarray(np.asarray(Wk)[sl, :].T).astype(np.float16),
            "wvT": np.ascontiguousarray(np.asarray(Wv)[sl, :].T).astype(np.float16),
            "woT": np.ascontiguousarray(np.asarray(Wo)[:, sl].T).astype(np.float16),
            "bq": np.ascontiguousarray(np.asarray(bq)[sl], dtype=np.float32),
        })
    return in_maps


_NC_CACHE = {}


def _get_nc(S=S_FULL):
    if S not in _NC_CACHE:
        _NC_CACHE[S] = build_kernel(S)
    return _NC_CACHE[S]


def kernel(x, Wq, bq, Wk, bk, Wv, bv, Wo, bo, _trace=False, _trace_cores=None):
    from concourse import bass_utils

    nc = _get_nc(S_FULL)
    in_maps = shard_inputs(x, Wq, bq, Wk, bk, Wv, bv, Wo, bo)
    kwargs = {}
    if _trace:
        kwargs = dict(trace=True, trace_cores=_trace_cores or [0])
    res = bass_utils.run_bass_kernel_spmd(
        nc, in_maps, core_ids=list(range(NCORES)), **kwargs)
    out = np.zeros((B, S_FULL, D), dtype=np.float32)
    for c in range(NCORES):
        out += res.results[c]["part"]
    # bv is folded out of the device kernel: softmax rows sum to one, so its
    # contribution to the output is the constant Wo @ bv. Add it with bo here.
    bias = (np.asarray(Wo, dtype=np.float64) @ np.asarray(bv, dtype=np.float64)
            + np.asarray(bo, dtype=np.float64))
    out += bias.astype(np.float32)[None, None, :]
    if _trace:
        kernel._last_results = res
    return out


# revision 39
# speedup vs baseline: 1.1302x; 1.0174x over previous
"""Trainium2 Bass kernel for nn_MemoryEfficientAttention (full MHA).

Reference computation (fp32):
    q = split_heads(x @ Wq.T + bq); k, v likewise
    attn = softmax(q @ k.T / sqrt(64))
    out = merge_heads(attn @ v) @ Wo.T + bo

Shapes: B=2, S=4096, D=1024, H=16, head_dim=64.

Sharding across 8 NeuronCores (Megatron attention sharding):
  - 2 heads per core (= 128 of the 1024 projection dims, contiguous slice).
  - Q/K/V projections column-parallel, output projection row-parallel;
    the 8 per-core partial outputs are summed on the host (+ bo).
  - bv never enters the device: softmax rows sum to 1, so its entire effect
    on the output is the constant vector Wo @ bv, added on the host.

Per-core kernel (flash-attention style, nothing S^2-sized touches HBM):
  The kernel is ScalarE-bound: the exp of the S^2 score matrix costs
  ~1.11us per (q-chunk, key-tile) step and 512 such steps = 569us.  All
  emission is therefore woven so ScalarE never starves: projection
  matmul chains are split in half and spread between attention j-steps
  (one chain-half per few steps), never as multi-us blocks.  xt is
  4-deep buffered so input DMA is never gated by tile release.
  Per j-step: scoresT for both heads run concurrently on row-split PE
  tiles, exp on ScalarE (no max subtraction -- scores are bounded), fp16
  PV matmul with an interleaved ones-column accumulating the softmax
  denominator.  Raw output + reciprocal of the denominator leave PSUM
  immediately; normalization resolves during the next chunk.  Output
  projection for chunk qc is woven into chunk qc+1's steps.
"""

import sys

if "/opt/trn_rl_repo" not in sys.path:
    sys.path.insert(0, "/opt/trn_rl_repo")

import numpy as np

B = 2
S_FULL = 4096
D = 1024
H = 16
HD = 64
NCORES = 8
DC = 128          # head dims per core (2 heads x 64)
SCALE = 1.0 / 8.0  # 1/sqrt(64)


def build_kernel(S=S_FULL):
    """Build the per-core Bass program. Returns the compiled Bacc object."""
    import concourse.bacc as bacc
    import concourse.tile as tile
    from concourse import mybir

    f32 = mybir.dt.float32
    f16 = mybir.dt.float16
    AF = mybir.ActivationFunctionType

    KT = D // 128       # k-tiles over the projection contraction dim
    SQ = 512            # q-chunk size
    NQC = S // SQ       # q chunks per batch
    NKT = S // 128      # key tiles per batch
    NM = S // 512       # x chunks for projections

    nc = bacc.Bacc("TRN2", target_bir_lowering=False, debug=False,
                   num_devices=NCORES)

    # All inputs are host-prepacked partition-major so every DMA is one
    # contiguous descriptor per partition (strided patterns cost 3-5us of
    # descriptor generation on the queue engines).
    NM_ = S // 512
    xP = nc.dram_tensor("xP", [B, NM_, 128, (D // 128) * 512], f16,
                        kind="ExternalInput").ap()
    wqT = nc.dram_tensor("wqT", [128, (D // 128) * DC], f16,
                         kind="ExternalInput").ap()
    wkT = nc.dram_tensor("wkT", [128, (D // 128) * DC], f16,
                         kind="ExternalInput").ap()
    wvT = nc.dram_tensor("wvT", [128, (D // 128) * DC], f16,
                         kind="ExternalInput").ap()
    woT = nc.dram_tensor("woT", [DC, D], f16, kind="ExternalInput").ap()
    bq = nc.dram_tensor("bq", [DC], f32, kind="ExternalInput").ap()
    part = nc.dram_tensor("part", [B, S, D], f32, kind="ExternalOutput").ap()

    with tile.TileContext(nc) as tc:
        with (
            tc.tile_pool(name="consts", bufs=1) as consts,
            tc.tile_pool(name="xt", bufs=4) as xt_pool,
            tc.tile_pool(name="qkv", bufs=2) as qkv_pool,
            tc.tile_pool(name="exp", bufs=4) as exp_pool,
            tc.tile_pool(name="exp_pf", bufs=16) as exp_pf_pool,
            tc.tile_pool(name="att", bufs=2) as att_pool,
            tc.tile_pool(name="small", bufs=4) as small_pool,
            tc.tile_pool(name="bounce", bufs=2, space="DRAM") as dram_pool,
            tc.tile_pool(name="ps_scores", bufs=2, space="PSUM") as ps_scores,
            tc.tile_pool(name="ps_acc", bufs=2, space="PSUM") as ps_acc,
            tc.tile_pool(name="ps_shared", bufs=2, space="PSUM") as ps_shared,
        ):
            # ---- constants ----
            wq_sb = consts.tile([128, KT, DC], f16)
            wk_sb = consts.tile([128, KT, DC], f16)
            wv_sb = consts.tile([128, KT, DC], f16)
            wo_sb = consts.tile([128, D], f16)
            bq_sb = consts.tile([128, 1], f32)

            # tiny dummy exp so the ACT table set loads during startup DMAs
            warm = consts.tile([128, 1], f32)
            nc.vector.memset(warm[:], 0.0)
            nc.scalar.activation(warm[:], warm[:], AF.Exp, scale=1.0)

            # DMA order is latency-critical: the first k-chain needs wk +
            # both xt halves, so wk leads the sync queue and the xt DMAs
            # (emitted right after, in phase A below) go next on both
            # queues; wq/bq follow on sync, wv/wo trail on gpsimd.  bk is
            # dropped entirely: it shifts every score by (q.bk)/8,
            # constant along the key axis, which cancels in the softmax.
            def load_consts_head():
                nc.sync.dma_start(
                    out=wk_sb[:],
                    in_=wkT.rearrange("p (kt m) -> p kt m", kt=KT))

            def load_consts_tail():
                nc.sync.dma_start(
                    out=wq_sb[:],
                    in_=wqT.rearrange("p (kt m) -> p kt m", kt=KT))
                nc.sync.dma_start(
                    out=bq_sb[:], in_=bq.rearrange("(p o) -> p o", o=1))
                nc.gpsimd.dma_start(
                    out=wv_sb[:],
                    in_=wvT.rearrange("p (kt m) -> p kt m", kt=KT))
                nc.gpsimd.dma_start(out=wo_sb[:], in_=woT)

            state = [None, None]  # per-batch dict of tiles

            def alloc_batch(b):
                qT_sb = qkv_pool.tile([128, S], f16, tag="qT", name=f"qT_{b}")
                kT_sb = qkv_pool.tile([128, S], f16, tag="kT", name=f"kT_{b}")
                # v natural layout, per key-tile: [vA(64) | 1 | vB(64) | 1]
                v_sb = qkv_pool.tile([128, NKT, 130], f16, tag="v",
                                     name=f"v_{b}")
                nc.vector.memset(v_sb[:, :, 64:65], 1.0)
                nc.vector.memset(v_sb[:, :, 129:130], 1.0)
                attT_sb = att_pool.tile([128, S], f16, tag="attT",
                                        name=f"attT_{b}")
                den_dram = dram_pool.tile([2, S], f16, tag="den",
                                          name=f"den_{b}")
                state[b] = dict(qT=qT_sb, kT=kT_sb, v=v_sb, attT=attT_sb,
                                den=den_dram)

            def proj_dma(b, m):
                """Allocate the x tile for chunk m and return its DMA item.
                Call order must match usage order (per-tag ring)."""
                xt = xt_pool.tile([128, KT, 512], f16, tag="xt",
                                  name=f"xt_{b}_{m}")

                def run():
                    xsrc = xP[b, m].rearrange("p (kt s) -> p kt s", kt=KT)
                    half = KT // 2
                    nc.sync.dma_start(out=xt[:, 0:half, :],
                                      in_=xsrc[:, 0:half, :])
                    nc.gpsimd.dma_start(out=xt[:, half:, :],
                                        in_=xsrc[:, half:, :])
                return xt, run

            def proj_chains(b, m, xt):
                """Projection matmul chains for chunk m as 8 small items
                (each <=1us of PE). PSUM tiles alloc at emission time so
                the shared-ring order matches usage order."""
                st = state[b]
                items = []

                def qk_chain(w_sb, b_sb, dst):
                    cell = {}

                    def half1():
                        cell["ps"] = ps_shared.tile(
                            [128, 512], f32, tag="shared",
                            name=f"ps_{b}_{m}_{id(w_sb)}")
                        for j in range(KT // 2):
                            nc.tensor.matmul(
                                cell["ps"][:], lhsT=w_sb[:, j, :],
                                rhs=xt[:, j, :],
                                start=(j == 0), stop=False)

                    def half2():
                        ps = cell["ps"]
                        for j in range(KT // 2, KT):
                            nc.tensor.matmul(
                                ps[:], lhsT=w_sb[:, j, :], rhs=xt[:, j, :],
                                start=False, stop=(j == KT - 1))
                        dslice = dst[:, m * 512:(m + 1) * 512]
                        if b_sb is None:
                            nc.vector.tensor_copy(dslice, ps[:])
                        else:
                            nc.vector.tensor_scalar_add(dslice, ps[:], b_sb[:])
                    return [half1, half2]

                items += qk_chain(wk_sb, None, st["kT"])
                items += qk_chain(wq_sb, bq_sb, st["qT"])

                # V in natural layout: x-tile stationary, Wv moving.
                # Split in halves like q/k so no woven item exceeds ~1us.
                def v_items(t):
                    cell = {}

                    def half1():
                        cell["ps"] = ps_shared.tile(
                            [128, 512], f32, tag="shared",
                            name=f"psv_{b}_{m}_{t}")
                        for j in range(KT // 2):
                            nc.tensor.matmul(
                                cell["ps"][:, 0:DC],
                                lhsT=xt[:, j, t * 128:(t + 1) * 128],
                                rhs=wv_sb[:, j, :],
                                start=(j == 0), stop=False)

                    def half2():
                        psv = cell["ps"]
                        for j in range(KT // 2, KT):
                            nc.tensor.matmul(
                                psv[:, 0:DC],
                                lhsT=xt[:, j, t * 128:(t + 1) * 128],
                                rhs=wv_sb[:, j, :],
                                start=False, stop=(j == KT - 1))
                        kt_idx = m * 4 + t
                        nc.vector.tensor_copy(
                            st["v"][:, kt_idx, 0:64], psv[:, 0:64])
                        nc.vector.tensor_copy(
                            st["v"][:, kt_idx, 65:129], psv[:, 64:128])
                    return [half1, half2]

                for t in range(4):
                    items += v_items(t)
                return items

            def attn_parts(b, qc, pf_tag=None):
                """One q-chunk of 512 rows, split into scores+exp parts and
                AV parts so emission can skew AV behind scores (the first
                AV of a chunk waits on the acc-bank release; skewing keeps
                it from blocking ready scores in the in-order PE queue).
                pf_tag: use a dedicated deep ex ring (phase-A prefetch)."""
                st = state[b]
                qT_sb, kT_sb, v_sb = st["qT"], st["kT"], st["v"]
                q0, q1 = qc * SQ, (qc + 1) * SQ
                ctx = {"ex": {}}

                def sc_part(j):
                    def run():
                        k0, k1 = j * 128, (j + 1) * 128
                        pss = ps_scores.tile([128, 2 * SQ], f32, tag="scores",
                                             name=f"pss_{b}_{qc}_{j}")
                        for hh in range(2):
                            nc.tensor.matmul(
                                pss[:, hh * SQ:(hh + 1) * SQ],
                                lhsT=kT_sb[hh * 64:(hh + 1) * 64, k0:k1],
                                rhs=qT_sb[hh * 64:(hh + 1) * 64, q0:q1],
                                start=True, stop=True)
                        pf = pf_tag and j < 16
                        pool = exp_pf_pool if pf else exp_pool
                        ex = pool.tile([128, 2 * SQ], f16,
                                       tag="exp_pf" if pf else "exp",
                                       name=f"ex_{b}_{qc}_{j}")
                        nc.scalar.activation(ex[:], pss[:], AF.Exp,
                                             scale=SCALE)
                        ctx["ex"][j] = ex
                    return run

                def av_part(j):
                    def run():
                        if j == 0:
                            ctx["acc"] = [
                                ps_acc.tile([128, SQ], f32, tag="acc",
                                            name=f"acc{hh}_{b}_{qc}")
                                for hh in range(2)]
                        accs = ctx["acc"]
                        ex = ctx["ex"].pop(j)
                        for hh in range(2):
                            nc.tensor.matmul(
                                accs[hh][0:65, :],
                                lhsT=v_sb[:, j, hh * 65:(hh + 1) * 65],
                                rhs=ex[:, hh * SQ:(hh + 1) * SQ],
                                start=(j == 0), stop=(j == NKT - 1))
                    return run

                return ([sc_part(j) for j in range(NKT)],
                        [av_part(j) for j in range(NKT)], ctx)

            def attn_fin(b, qc, ctx, use_scalar=False):
                """PSUM release + softmax normalization for a q-chunk.
                use_scalar: offload one head's release copies to ScalarE
                (only safe after the last exp has been emitted)."""
                st = state[b]
                attT_sb, den_dram = st["attT"], st["den"]
                q0, q1 = qc * SQ, (qc + 1) * SQ
                accs = ctx["acc"]
                for hh in range(2):
                    if use_scalar and hh == 1:
                        nc.scalar.copy(
                            attT_sb[hh * 64:(hh + 1) * 64, q0:q1],
                            accs[hh][0:64, :])
                    else:
                        nc.vector.tensor_copy(
                            attT_sb[hh * 64:(hh + 1) * 64, q0:q1],
                            accs[hh][0:64, :])
                    dn = small_pool.tile([1, SQ], f32, tag="dn",
                                         name=f"dn_{b}_{qc}_{hh}")
                    nc.vector.tensor_copy(dn[:], accs[hh][64:65, :])
                    rcp = small_pool.tile([1, SQ], f32, tag="rcp",
                                          name=f"rcp_{b}_{qc}_{hh}")
                    nc.vector.reciprocal_approx_fast(rcp[:], dn[:])
                    rcp16 = small_pool.tile([1, SQ], f16, tag="rcp16",
                                            name=f"rcp16_{b}_{qc}_{hh}")
                    with nc.allow_low_precision(reason="fp16 softmax recip"):
                        nc.vector.tensor_copy(rcp16[:], rcp[:])
                    nc.gpsimd.dma_start(out=den_dram[hh, q0:q1], in_=rcp16[:])

                # broadcast the reciprocal + normalize; resolves during the
                # next chunk's attention
                for hh in range(2):
                    bc = small_pool.tile([128, SQ], f16, tag="bcast",
                                         name=f"bc_{b}_{qc}_{hh}")
                    bch = bc[hh * 64:(hh + 1) * 64, :]
                    rd = den_dram[hh, q0:q1]
                    bcast_src = rd.__class__(
                        tensor=rd.tensor, offset=rd.offset,
                        ap=[[0, 64]] + list(rd.ap),
                    )
                    nc.gpsimd.dma_start(out=bch, in_=bcast_src)
                    nc.vector.tensor_mul(
                        attT_sb[hh * 64:(hh + 1) * 64, q0:q1],
                        attT_sb[hh * 64:(hh + 1) * 64, q0:q1],
                        bch,
                    )

            def tail_items(b, qc, engines=("vector",)):
                """Output projection for a q-chunk as 8 single-tile items,
                emitted one chunk late so the normalization chain has
                resolved and the in-order PE never stalls on it."""
                st = state[b]
                attT_sb = st["attT"]
                q0 = qc * SQ
                items = []
                for sti in range(SQ // 128):
                    for oc in range(D // 512):
                        def run(sti=sti, oc=oc, n=sti * 2 + oc):
                            s0 = q0 + sti * 128
                            s1 = s0 + 128
                            pso = ps_shared.tile(
                                [128, 512], f32, tag="shared",
                                name=f"pso_{b}_{qc}_{sti}_{oc}")
                            nc.tensor.matmul(
                                pso[:], lhsT=attT_sb[:, s0:s1],
                                rhs=wo_sb[:, oc * 512:(oc + 1) * 512],
                                start=True, stop=True)
                            ob = small_pool.tile([128, 512], f32, tag="ob",
                                                 name=f"ob_{b}_{qc}_{sti}_{oc}")
                            eng = engines[n % len(engines)]
                            if eng == "scalar":
                                nc.scalar.copy(ob[:], pso[:])
                            else:
                                nc.vector.tensor_copy(ob[:], pso[:])
                            nc.sync.dma_start(
                                out=part[b, s0:s1, oc * 512:(oc + 1) * 512],
                                in_=ob[:])
                        items.append(run)
                return items

            def weave(steps, extras):
                """Run all steps in order, spreading extras evenly between
                them (extras trail: first extra after the first step)."""
                n, e = len(steps), len(extras)
                ei = 0
                for i, s in enumerate(steps):
                    s()
                    while ei < e and (ei + 1) * n <= (i + 1) * e:
                        extras[ei]()
                        ei += 1
                while ei < e:
                    extras[ei]()
                    ei += 1

            def mk_slot(*fns):
                def run():
                    for f in fns:
                        f()
                return run

            def window(sc, av, carry):
                """Stitched slot window for a standard chunk: carry (the
                previous chunk's fin) lands at slot 0, av lags sc by 2,
                av30/av31 run in a final sc-free slot.  Returns (slots,
                carry_out_prefix) where the caller appends fin."""
                slots = [mk_slot(sc[0], *carry), mk_slot(sc[1])]
                for j in range(2, NKT):
                    slots.append(mk_slot(sc[j], av[j - 2]))
                slots.append(mk_slot(av[NKT - 2], av[NKT - 1]))
                return slots

            # ---- emission schedule ----
            # PE warmup: ~16 dummy matmuls ramp the PE out of its low
            # p-state (0.65GHz cold, 2.4GHz after ~3us of execution) while
            # the startup DMAs are in flight, so the first projection
            # chains run at speed.
            load_consts_head()
            dummy = consts.tile([128, 256], f16)
            nc.vector.memset(dummy[:], 0.0)
            for w in range(26):
                psw = ps_shared.tile([128, 512], f32, tag="shared",
                                     name=f"warmmm_{w}")
                nc.tensor.matmul(psw[:, 0:256], lhsT=dummy[:, 0:128],
                                 rhs=dummy[:], start=True, stop=True)

            # Phase A: batch-0 chunk-0 k/q chains up front, then chunks
            # 1..7 woven with the first q-chunk's slots (4 per chunk, one
            # chunk-DMA ahead).  qc1's first PF score+exp parts are
            # prefetched into phase A (deep ex ring) so ScalarE has more
            # than one chunk's exp work while PE grinds projections.
            PF = 16
            alloc_batch(0)
            xts = {}
            xts[0], dma = proj_dma(0, 0)
            dma()
            load_consts_tail()
            xts[1], dma = proj_dma(0, 1)
            dma()
            ch0 = proj_chains(0, 0, xts[0])
            for it in ch0[:4]:
                it()
            sc0, av0, ctx0 = attn_parts(0, 0)
            sc1, av1, ctx1 = attn_parts(0, 1, pf_tag="exp_pf")
            pf_sched = {2: [0, 1], 3: [2, 3], 4: [4, 5], 5: [6, 7, 8],
                        6: [9, 10, 11], 7: [12, 13, 14, 15]}

            def slot_a(j):
                def run():
                    sc0[j]()
                    if j >= 2:
                        av0[j - 2]()
                return run

            for m in range(1, NM):
                extras = []
                if m == 1:
                    extras += ch0[4:]
                if m + 1 < NM:
                    xts[m + 1], dma = proj_dma(0, m + 1)
                    extras.append(dma)
                extras += proj_chains(0, m, xts[m])
                extras += [sc1[p] for p in pf_sched.get(m, [])]
                weave([slot_a(j) for j in range(4 * (m - 1), 4 * m)], extras)
            for j in range(4 * (NM - 1), NKT):
                slot_a(j)()
            mk_slot(av0[NKT - 2], av0[NKT - 1])()
            carry = [lambda: attn_fin(0, 0, ctx0)]
            pending = [(0, 0)]

            # Phase B: batch-0 qc1 first catches up the prefetched AVs
            # (its sc j>=8 allocations require av[j-4] emitted first),
            # then qc 2..7 with batch-1 projections woven over qc 2..6
            # and tails (2 chunks behind) woven throughout.
            def qc1_slots(carry):
                # sc j>=PF uses the normal ex ring (4): sc[j] allocation
                # waits on av[j-4], so every av[j-4] must be emitted before
                # sc[j].  av[0..PF-1] read the deep prefetch ring; the AVs
                # catch up in j-order while the remaining scores trickle.
                slots = [mk_slot(sc1[16], *carry), mk_slot(sc1[17]),
                         mk_slot(sc1[18], av1[0], av1[1]),
                         mk_slot(sc1[19], av1[2], av1[3]),
                         mk_slot(av1[4], av1[5]),
                         mk_slot(av1[6], av1[7]),
                         mk_slot(av1[8], av1[9]),
                         mk_slot(av1[10], av1[11]),
                         mk_slot(av1[12], av1[13]),
                         mk_slot(av1[14], av1[15], av1[16]),
                         mk_slot(sc1[20], av1[17]),
                         mk_slot(sc1[21], av1[18]),
                         mk_slot(sc1[22], av1[19], av1[20]),
                         mk_slot(sc1[23], av1[21])]
                for j in range(24, NKT):
                    slots.append(mk_slot(sc1[j], av1[j - 2]))
                slots.append(mk_slot(av1[NKT - 2], av1[NKT - 1]))
                return slots

            # Phase B carries no output-projection tails: its PE budget is
            # already full with batch-1 projections (attn 27us + proj 9us
            # vs ScalarE's 35.5us per chunk).  All tails drain in phase C,
            # which has the PE slack (2 per chunk).
            b1_items = []
            for qc in range(1, NQC):
                if qc == 1:
                    slots = qc1_slots(carry)
                    ctx = ctx1
                else:
                    sc, av, ctx = attn_parts(0, qc)
                    slots = window(sc, av, carry)
                if qc == 2:
                    alloc_batch(1)
                    b1_xts = {}
                    b1_xts[0], dma0 = proj_dma(1, 0)
                    b1_items.append(dma0)
                    for m in range(NM):
                        if m + 1 < NM:
                            b1_xts[m + 1], dma = proj_dma(1, m + 1)
                            b1_items.append(dma)
                        b1_items += proj_chains(1, m, b1_xts[m])
                    per_qc = (len(b1_items) + NQC - 4) // (NQC - 3)
                extras = []
                if b1_items:
                    take, b1_items = b1_items[:per_qc], b1_items[per_qc:]
                    extras += take
                weave(slots, extras)
                carry = [lambda ctx=ctx, qc=qc: attn_fin(0, qc, ctx)]
                pending.append((0, qc))

            # Phase C: batch-1 attention, draining two tails per chunk.
            for qc in range(NQC):
                sc, av, ctx = attn_parts(1, qc)
                extras = []
                for _ in range(2):
                    if len(pending) > 2:
                        extras += tail_items(*pending.pop(0))
                weave(window(sc, av, carry), extras)
                last = (qc == NQC - 1)
                carry = [lambda ctx=ctx, qc=qc, last=last: attn_fin(
                    1, qc, ctx, use_scalar=last)]
                pending.append((1, qc))

            # Final drain: the last fin runs under the second-to-last
            # chunk's output projection; ScalarE helps with copies and
            # ps_shared double-buffers the pso tiles.
            for f in carry:
                f()
            while pending:
                for it in tail_items(*pending.pop(0),
                                     engines=("scalar", "vector")):
                    it()

    nc.compile()
    return nc


def _pack_w(w):
    # [D, DC] -> [128, KT*DC] partition-major: row p holds w[kt*128+p, :]
    # for kt = 0..KT-1, so the weight DMA is one contiguous run/partition.
    kt = D // 128
    return np.ascontiguousarray(
        w.reshape(kt, 128, DC).transpose(1, 0, 2).reshape(128, kt * DC)
    ).astype(np.float16)


def shard_inputs(x, Wq, bq, Wk, bk, Wv, bv, Wo, bo, S=S_FULL):
    """Host-side sharding: returns list of 8 per-core input dicts."""
    x = np.asarray(x, dtype=np.float32)
    kt, nm = D // 128, S // 512
    # [B, S, D] -> [B, M, 128, KT*512] partition-major per x-chunk
    xP = np.ascontiguousarray(
        x.transpose(0, 2, 1)            # [B, D, S]
        .reshape(B, kt, 128, nm, 512)   # D=(kt p), S=(m s)
        .transpose(0, 3, 2, 1, 4)       # [B, M, 128, KT, 512]
        .reshape(B, nm, 128, kt * 512)
    ).astype(np.float16)
    in_maps = []
    for c in range(NCORES):
        sl = slice(c * DC, (c + 1) * DC)
        in_maps.append({
            "xP": xP,
            "wqT": _pack_w(np.asarray(Wq)[sl, :].T),
            "wkT": _pack_w(np.asarray(Wk)[sl, :].T),
            "wvT": _pack_w(np.asarray(Wv)[sl, :].T),
            "woT": np.ascontiguousarray(np.asarray(Wo)[:, sl].T).astype(np.float16),
            "bq": np.ascontiguousarray(np.asarray(bq)[sl], dtype=np.float32),
        })
    return in_maps


_NC_CACHE = {}


def _get_nc(S=S_FULL):
    if S not in _NC_CACHE:
        _NC_CACHE[S] = build_kernel(S)
    return _NC_CACHE[S]


def kernel(x, Wq, bq, Wk, bk, Wv, bv, Wo, bo, _trace=False, _trace_cores=None):
    from concourse import bass_utils

    nc = _get_nc(S_FULL)
    in_maps = shard_inputs(x, Wq, bq, Wk, bk, Wv, bv, Wo, bo)
    kwargs = {}
    if _trace:
        kwargs = dict(trace=True, trace_cores=_trace_cores or [0])
    res = bass_utils.run_bass_kernel_spmd(
        nc, in_maps, core_ids=list(range(NCORES)), **kwargs)
    out = np.zeros((B, S_FULL, D), dtype=np.float32)
    for c in range(NCORES):
        out += res.results[c]["part"]
    # bv is folded out of the device kernel: softmax rows sum to one, so its
    # contribution to the output is the constant Wo @ bv. Add it with bo here.
    bias = (np.asarray(Wo, dtype=np.float64) @ np.asarray(bv, dtype=np.float64)
            + np.asarray(bo, dtype=np.float64))
    out += bias.astype(np.float32)[None, None, :]
    if _trace:
        kernel._last_results = res
    return out
